# revision 1
# baseline (speedup 1.0000x reference)
"""Trainium2 Bass kernel for the sparse-conv network (nn_ExampleNet).

Pipeline (per batch image): scatter 200k sparse voxel features into a dense
[256,256,32] grid, SparseConv(32->64) + 2x SubMConv(64) with an active-site
mask, SparseConvTranspose(64, stride 2), dense 3x3 VALID conv -> [511,511,64].

Strategy: 8-way SPMD (4 batches x 2 row-halves). Host does the sparse->dense
scatter, mask dilations and data layout; each NeuronCore runs the fused
5-conv stack on its half-image in 16-row chunks, convs expressed as
shifted-window fp32r matmuls (taps packed in pairs across 128 partitions).
"""
from contextlib import ExitStack

import numpy as np
import ml_dtypes

import concourse.bacc as bacc
import concourse.mybir as mybir
import concourse.tile as tile
from concourse.bass_utils import run_bass_kernel_spmd

F32 = mybir.dt.float32
F32R = mybir.dt.float32r
BF16 = mybir.dt.bfloat16
RELU = mybir.ActivationFunctionType.Relu

B, H, W = 4, 256, 256
WP = W + 2          # padded width for x/h1-3 stores
CH = 16             # output rows per chunk
NCH = 16            # chunks per core
PITCH4 = 516        # h4 row pitch (513 cols + pad)
RXS = 138           # x slab rows
RM1 = 136           # mask1 slab rows
RM4 = 260           # mask4 slab rows

BLOCKS1 = [(0, 0), (0, 2), (2, 0), (2, 2)]
BLOCKS6 = [(d, x) for d in range(3) for x in (0, 2)]

_CACHE = {}


def _host_prep(features, coors, w1, b1, w2, b2, w3, b3, wt, bt, w5, b5):
    f32 = np.float32
    bi, yi, xi = coors[:, 0], coors[:, 1], coors[:, 2]
    flat = (bi.astype(np.int64) * H + yi) * W + xi
    dense = np.zeros((B * H * W, 32), f32)
    for c in range(32):
        dense[:, c] = np.bincount(flat, weights=features[:, c],
                                  minlength=B * H * W)
    dense = dense.reshape(B, H, W, 32)
    occ = np.bincount(flat, minlength=B * H * W).reshape(B, H, W) > 0
    m0p = np.zeros((B, H + 2, W + 2), bool)
    m0p[:, 1:-1, 1:-1] = occ
    m1 = np.zeros((B, H, W), bool)
    for dy in range(3):
        for dx in range(3):
            m1 |= m0p[:, dy:dy + H, dx:dx + W]
    m4 = np.zeros((B, 2 * H + 1, 2 * W + 1), bool)
    for dy in range(3):
        for dx in range(3):
            m4[:, dy:dy + 2 * H - 1:2, dx:dx + 2 * W - 1:2] |= m1

    wt_eff = wt[::-1, ::-1]  # jax conv_transpose applies the flipped kernel
    z32 = np.zeros((32, 64), f32)
    z64 = np.zeros((64, 64), f32)
    wc1 = np.zeros((128, 4, 64), f32)
    for i, (dy, dx) in enumerate(BLOCKS1):
        for g, (jy, jx) in enumerate([(0, 0), (0, 1), (1, 0), (1, 1)]):
            tap = w1[dy + jy, dx + jx] if (dy + jy < 3 and dx + jx < 3) else z32
            wc1[32 * g:32 * g + 32, i] = tap

    def mk6(w):
        out = np.zeros((128, 6, 64), f32)
        for i, (dy, dxb) in enumerate(BLOCKS6):
            out[0:64, i] = w[dy, dxb]
            out[64:128, i] = w[dy, dxb + 1] if dxb + 1 < 3 else z64
        return out

    wc2, wc3, wc5 = mk6(w2), mk6(w3), mk6(w5)
    wct = np.zeros((128, 6, 64), f32)
    wct[0:64, 0], wct[64:128, 0] = wt_eff[0, 2], wt_eff[0, 0]
    wct[0:64, 1], wct[64:128, 1] = wt_eff[2, 2], wt_eff[2, 0]
    wct[0:64, 2] = wt_eff[0, 1]
    wct[0:64, 3] = wt_eff[2, 1]
    wct[0:64, 4], wct[64:128, 4] = wt_eff[1, 2], wt_eff[1, 0]
    wct[0:64, 5] = wt_eff[1, 1]
    biases = np.stack([b1, b2, b3, bt, b5], 1).astype(f32)

    in_maps = []
    for core in range(8):
        b, half = core // 2, core % 2
        A0 = 0 if half == 0 else 128
        U0 = 2 * A0
        xs = np.zeros((32, RXS, WP), f32)
        lo, hi = max(0, A0 - 4), min(H, A0 - 4 + RXS)
        xs[:, lo - (A0 - 4):hi - (A0 - 4), 1:W + 1] = \
            dense[b, lo:hi].transpose(2, 0, 1)
        ms = np.zeros((RM1, WP), ml_dtypes.bfloat16)
        lo, hi = max(0, A0 - 3), min(H, A0 - 3 + RM1)
        ms[lo - (A0 - 3):hi - (A0 - 3), 1:W + 1] = m1[b, lo:hi]
        m4s = np.zeros((RM4, PITCH4), ml_dtypes.bfloat16)
        lo, hi = max(0, U0), min(2 * H + 1, U0 + RM4)
        m4s[lo - U0:hi - U0, :2 * W + 1] = m4[b, lo:hi]
        xsf = xs.reshape(32, -1)
        xs4 = np.zeros((128, RXS * WP), f32)
        for g, sh in enumerate([0, 1, WP, WP + 1]):
            xs4[32 * g:32 * g + 32, :RXS * WP - sh] = xsf[:, sh:]
        in_maps.append(dict(
            xs=np.ascontiguousarray(xs4),
            ms=np.ascontiguousarray(np.broadcast_to(ms[None], (64, RM1, WP))),
            m4s=np.ascontiguousarray(
                np.broadcast_to(m4s[None], (64, RM4, PITCH4))),
            wc1=wc1, wc2=wc2, wc3=wc3, wct=wct, wc5=wc5, biases=biases,
        ))
    return in_maps


def _build_program():
    nc = bacc.Bacc("TRN2", target_bir_lowering=False, debug=False,
                   enable_asserts=True, num_devices=8)

    xs_d = nc.dram_tensor("xs", [128, RXS, WP], F32R,
                          kind="ExternalInput").ap()
    ms_d = nc.dram_tensor("ms", [64, RM1, WP], BF16, kind="ExternalInput").ap()
    m4_d = nc.dram_tensor("m4s", [64, RM4, PITCH4], BF16,
                          kind="ExternalInput").ap()
    wc1_d = nc.dram_tensor("wc1", [128, 4, 64], F32R, kind="ExternalInput").ap()
    wc2_d = nc.dram_tensor("wc2", [128, 6, 64], F32R, kind="ExternalInput").ap()
    wc3_d = nc.dram_tensor("wc3", [128, 6, 64], F32R, kind="ExternalInput").ap()
    wct_d = nc.dram_tensor("wct", [128, 6, 64], F32R, kind="ExternalInput").ap()
    wc5_d = nc.dram_tensor("wc5", [128, 6, 64], F32R, kind="ExternalInput").ap()
    bias_d = nc.dram_tensor("biases", [64, 5], F32, kind="ExternalInput").ap()
    out_d = nc.dram_tensor("out", [64, 256 * 511], F32,
                           kind="ExternalOutput").ap()

    with tile.TileContext(nc) as tc, ExitStack() as ctx:
        wp = ctx.enter_context(tc.tile_pool(name="wp", bufs=1))
        xp = ctx.enter_context(tc.tile_pool(name="xp", bufs=2))
        mp = ctx.enter_context(tc.tile_pool(name="mp", bufs=2))
        hp = ctx.enter_context(tc.tile_pool(name="hp", bufs=1))
        pp = ctx.enter_context(tc.tile_pool(name="pp", bufs=2, space="PSUM"))
        op = ctx.enter_context(tc.tile_pool(name="op", bufs=4))

        w1t = wp.tile([128, 4, 64], F32R, name="w1t")
        w2t = wp.tile([128, 6, 64], F32R, name="w2t")
        w3t = wp.tile([128, 6, 64], F32R, name="w3t")
        wtt = wp.tile([128, 6, 64], F32R, name="wtt")
        w5t = wp.tile([128, 6, 64], F32R, name="w5t")
        bt = wp.tile([64, 5], F32, name="bt")
        nc.sync.dma_start(w1t[:], wc1_d[:])
        nc.sync.dma_start(bt[:], bias_d[:])
        nc.scalar.dma_start(w2t[:], wc2_d[:])
        nc.scalar.dma_start(w3t[:], wc3_d[:])
        nc.gpsimd.dma_start(wtt[:], wct_d[:])
        nc.gpsimd.dma_start(w5t[:], wc5_d[:])

        def conv_layer(inp, wt_, blocks, nrows, bias_ap, m_ch, moff, h_out):
            n = nrows * WP
            half = (nrows // 2 + 1) & ~1   # even row count, covers split row
            sp_ = (half - 1) * WP          # split point: first copy needs
            lo = h_out[0:64].rearrange("p r c -> p (r c)")   # rows < half
            hi = h_out[64:128].rearrange("p r c -> p (r c)")
            for j in range(0, nrows, 2):
                pc = pp.tile([64, 2, 256], F32, name="pc", tag="pc", bufs=3)
                for i, (dy, dx) in enumerate(blocks):
                    nc.tensor.matmul(
                        pc[:], wt_[:, i, :], inp[:, j + dy:j + dy + 2,
                                                 dx:dx + 256],
                        start=(i == 0), stop=(i == len(blocks) - 1))
                dst = h_out[0:64, j:j + 2, 1:257]
                nc.scalar.activation(dst, pc[:], RELU, bias=bias_ap)
                nc.vector.tensor_mul(dst, dst,
                                     m_ch[0:64, j + moff:j + moff + 2, 1:257])
                if j + 2 == half:
                    nc.vector.tensor_copy(hi[:, 0:sp_], lo[:, 1:sp_ + 1])
            nc.vector.tensor_copy(hi[:, sp_:n - 1], lo[:, sp_ + 1:n])

        def load_chunk(c):
            x_ch = xp.tile([128, CH, WP], F32R, name="x_ch", tag="x")
            nc.sync.dma_start(x_ch[:, 0:8, :], xs_d[:, 8 * c:8 * c + 8, :])
            nc.sync.dma_start(x_ch[:, 8:CH, :],
                              xs_d[:, 8 * c + 8:8 * c + CH, :])
            m1_ch = mp.tile([64, 14, WP], BF16, name="m1_ch", tag="m1")
            nc.sync.dma_start(m1_ch[:], ms_d[:, 8 * c:8 * c + 14, :])
            m4_ch = mp.tile([64, 18, PITCH4], BF16, name="m4_ch", tag="m4")
            nc.sync.dma_start(m4_ch[:], m4_d[:, 16 * c:16 * c + 18, :])
            return x_ch, m1_ch, m4_ch

        def emit_convT(h3, h4, m4_ch):
            for la in range(9):
                # even-col planes (N=258, odd width): one a-row per matmul
                p00 = pp.tile([64, 258], F32, name="p00", tag="pT", bufs=3)
                nc.tensor.matmul(p00[:], wtt[:, 0, :], h3[:, la + 1, 0:258],
                                 start=True, stop=False)
                nc.tensor.matmul(p00[:], wtt[:, 1, :], h3[:, la, 0:258],
                                 start=False, stop=True)
                p10 = pp.tile([64, 258], F32, name="p10", tag="pT", bufs=3)
                nc.tensor.matmul(p10[:], wtt[:, 4, :], h3[:, la + 1, 0:258],
                                 start=True, stop=True)
                u = 2 * la
                de = h4[0:64, u, 0:513:2]
                nc.scalar.activation(de, p00[:, 0:257], RELU, bias=bt[:, 3:4])
                nc.vector.tensor_mul(de, de, m4_ch[0:64, u, 0:513:2])
                de = h4[0:64, u + 1, 0:513:2]
                nc.scalar.activation(de, p10[:, 0:257], RELU, bias=bt[:, 3:4])
                nc.vector.tensor_mul(de, de, m4_ch[0:64, u + 1, 0:513:2])
            # odd-col planes (N=256): batch two a-rows per matmul
            for la in range(0, 9, 2):
                nr = 2 if la + 1 < 9 else 1
                p01 = pp.tile([64, 2, 256], F32, name="p01", tag="pT", bufs=3)
                nc.tensor.matmul(p01[:, 0:nr], wtt[:, 2, :],
                                 h3[:, la + 1:la + 1 + nr, 1:257],
                                 start=True, stop=False)
                nc.tensor.matmul(p01[:, 0:nr], wtt[:, 3, :],
                                 h3[:, la:la + nr, 1:257],
                                 start=False, stop=True)
                p11 = pp.tile([64, 2, 256], F32, name="p11", tag="pT", bufs=3)
                nc.tensor.matmul(p11[:, 0:nr], wtt[:, 5, :],
                                 h3[:, la + 1:la + 1 + nr, 1:257],
                                 start=True, stop=True)
                for pu, pz in [(0, p01), (1, p11)]:
                    do = h4[0:64, 2 * la + pu:2 * la + pu + 2 * nr - 1:2, 1:513:2]
                    nc.scalar.activation(do, pz[:, 0:nr], RELU, bias=bt[:, 3:4])
                    nc.vector.tensor_mul(
                        do, do,
                        m4_ch[0:64, 2 * la + pu:2 * la + pu + 2 * nr - 1:2, 1:513:2])
                if la == 4:
                    nc.vector.tensor_copy(
                        h4[64:128].rearrange("p r c -> p (r c)")[:,
                                                                 0:10 * PITCH4],
                        h4[0:64].rearrange("p r c -> p (r c)")[:,
                                                               1:10 * PITCH4 + 1])
            n4 = 18 * PITCH4
            s4 = 10 * PITCH4
            lo4 = h4[0:64].rearrange("p r c -> p (r c)")
            hi4 = h4[64:128].rearrange("p r c -> p (r c)")
            nc.vector.tensor_copy(hi4[:, s4:n4 - 1], lo4[:, s4 + 1:n4])

        def emit_conv5(h4, c, jos):
            for jo in jos:
                p5 = pp.tile([64, 512], F32, name="p5", tag="p5")
                for i, (dy, dxb) in enumerate(BLOCKS6):
                    nc.tensor.matmul(p5[:], w5t[:, i, :],
                                     h4[:, jo + dy, dxb:dxb + 512],
                                     start=(i == 0), stop=(i == 5))
                out_sb = op.tile([64, 511], F32, name="out_sb", tag="o", bufs=6)
                nc.scalar.activation(out_sb[:], p5[:, 0:511], RELU,
                                     bias=bt[:, 4:5])
                nc.gpsimd.dma_start(
                    out_d[:, (16 * c + jo) * 511:(16 * c + jo + 1) * 511],
                    out_sb[:])

        prev = None  # (h3, h4, m4_ch, c)
        for it in range(NCH + 1):
            cur = None
            if it < NCH:
                x_ch, m1_ch, m4_ch = load_chunk(it)
                h1 = hp.tile([128, 14, WP], F32R, name="h1", tag="h1")
                h2 = hp.tile([128, 12, WP], F32R, name="h2", tag="h2")
                h3 = hp.tile([128, 10, WP], F32R, name="h3", tag="h3", bufs=2)
                for h_ in (h1, h2, h3):
                    nc.gpsimd.memset(h_[0:128, :, 0:1].bitcast(F32), 0)
                    nc.gpsimd.memset(h_[0:128, :, 257:258].bitcast(F32), 0)
                    nc.gpsimd.memset(h_[64:128, :, 256:257].bitcast(F32), 0)
                conv_layer(x_ch, w1t, BLOCKS1, 14, bt[:, 0:1], m1_ch, 0, h1)
                cur = (h3, m4_ch)
            if prev is not None:
                h3p, h4p, m4p, cp = prev
                emit_convT(h3p, h4p, m4p)
            if it < NCH:
                conv_layer(h1, w2t, BLOCKS6, 12, bt[:, 1:2], m1_ch, 1, h2)
            if prev is not None:
                emit_conv5(h4p, cp, range(0, 8))
            if it < NCH:
                conv_layer(h2, w3t, BLOCKS6, 10, bt[:, 2:3], m1_ch, 2, h3)
            if prev is not None:
                emit_conv5(h4p, cp, range(8, 16))
            if it < NCH:
                h4 = hp.tile([128, 18, PITCH4], F32R, name="h4", tag="h4")
                nc.gpsimd.memset(h4[64:128, :, 512:513].bitcast(F32), 0)
                nc.gpsimd.memset(h4[0:128, :, 513:516].bitcast(F32), 0)
                prev = (h3, h4, m4_ch, it)
            else:
                prev = None

    nc.compile()
    return nc


def kernel(**inputs):
    features = np.asarray(inputs["features"], np.float32)
    coors = np.asarray(inputs["coors"], np.int32)
    args = [np.asarray(inputs[k], np.float32) for k in
            ("w1", "b1", "w2", "b2", "w3", "b3", "wt", "bt", "w5", "b5")]
    in_maps = _host_prep(features, coors, *args)
    if "nc" not in _CACHE:
        _CACHE["nc"] = _build_program()
    res = run_bass_kernel_spmd(_CACHE["nc"], in_maps,
                               core_ids=list(range(8)), trace=False)
    full = np.zeros((B, 511, 511, 64), np.float32)
    for core in range(8):
        o = res.results[core]["out"].reshape(64, 256, 511)
        b, half = core // 2, core % 2
        if half == 0:
            full[b, 0:256] = o.transpose(1, 2, 0)
        else:
            full[b, 256:511] = o[:, 0:255].transpose(1, 2, 0)
    return full



# revision 11
# speedup vs baseline: 1.7748x; 1.7748x over previous
"""Trainium2 Bass kernel for the sparse-conv network (nn_ExampleNet).

Pipeline (per batch image): scatter 200k sparse voxel features into a dense
[256,256,32] grid, SparseConv(32->64) + 2x SubMConv(64) with an active-site
mask, SparseConvTranspose(64, stride 2), dense 3x3 VALID conv -> [511,511,64].

Strategy: 8-way SPMD (4 batches x 2 row-halves), row-parity-packed layout:
every on-chip tensor stores row pairs across the 128 partitions
(partitions 0:64 = even row channels, 64:128 = odd row), so every matmul
uses all 128 PE output lanes and every activation/mask-multiply runs at
128-partition throughput. Everything stored in bf16 (halves SBUF + DMA +
doubles DVE rate); PSUM accumulation stays fp32.
"""
from contextlib import ExitStack

import numpy as np
import ml_dtypes

import concourse.bacc as bacc
import concourse.mybir as mybir
import concourse.tile as tile
from concourse.bass_utils import run_bass_kernel_spmd

F32 = mybir.dt.float32
F32R = mybir.dt.float32r
BF16 = mybir.dt.bfloat16
RELU = mybir.ActivationFunctionType.Relu
BF = ml_dtypes.bfloat16

B, H, W = 4, 256, 256
NCOL = 258      # padded col count for x/h1-3 slabs
P4 = 516        # h4/m4 col pitch
NCH = 8         # chunks per core (32 output rows each)

DELTAS1 = [(-1, 0), (-1, 2), (0, 0), (0, 2), (1, 0), (1, 2)]
DELTAS9 = [(dlt, d) for dlt in (-1, 0, 1) for d in range(3)]
SIGD = [(sg, d) for sg in (0, 1) for d in range(3)]
# convT up-pair batches: (first local up-pair, count); even-l use the
# odd-aligned h3 copy, odd-l the standard one
TBATCH = [(0, 2), (4, 2), (8, 2), (12, 2), (16, 1),
          (1, 2), (5, 2), (9, 2), (13, 2)]

_CACHE = {}


def _pack_weights(w1, w2, w3, wt, w5):
    f32 = np.float32
    w1c = np.zeros((128, 6, 128), f32)
    for k, (dlt, d) in enumerate(DELTAS1):
        for rp in range(2):
            for cs in range(2):
                for op in range(2):
                    dy = 2 * dlt + rp - op + 1
                    dx = d + cs
                    if 0 <= dy < 3 and 0 <= dx < 3:
                        w1c[32 * cs + 64 * rp:32 * cs + 64 * rp + 32,
                            k, 64 * op:64 * op + 64] = w1[dy, dx]

    def pack9(wm):
        out = np.zeros((128, 9, 128), f32)
        for k, (dlt, d) in enumerate(DELTAS9):
            for rp in range(2):
                for op in range(2):
                    dy = 2 * dlt + rp - op + 1
                    if 0 <= dy < 3:
                        out[64 * rp:64 * rp + 64, k,
                            64 * op:64 * op + 64] = wm[dy, d]
        return out

    wte = wt[::-1, ::-1]  # jax conv_transpose applies the flipped kernel
    wtc = np.zeros((128, 3, 128), f32)
    for k, dx in enumerate((0, 2, 1)):
        wtc[0:64, k, 0:64] = wte[2, dx]
        wtc[64:128, k, 0:64] = wte[0, dx]
        wtc[64:128, k, 64:128] = wte[1, dx]
    w5c = np.zeros((128, 6, 128), f32)
    for k, (sg, d) in enumerate(SIGD):
        for rp in range(2):
            for op in range(2):
                dy = 2 * sg + rp - op
                if 0 <= dy < 3:
                    w5c[64 * rp:64 * rp + 64, k,
                        64 * op:64 * op + 64] = w5[dy, d]
    return tuple(w.astype(BF) for w in
                 (w1c, pack9(w2), pack9(w3), wtc, w5c))


def _host_prep(features, coors, w1, b1, w2, b2, w3, b3, wt, bt, w5, b5):
    f32 = np.float32
    bi, yi, xi = coors[:, 0], coors[:, 1], coors[:, 2]
    flat = (bi.astype(np.int64) * H + yi) * W + xi
    dense = np.zeros((B * H * W, 32), f32)
    for c in range(32):
        dense[:, c] = np.bincount(flat, weights=features[:, c],
                                  minlength=B * H * W)
    dense = dense.reshape(B, H, W, 32)
    occ = np.bincount(flat, minlength=B * H * W).reshape(B, H, W) > 0
    m0p = np.zeros((B, H + 2, W + 2), bool)
    m0p[:, 1:-1, 1:-1] = occ
    m1 = np.zeros((B, H, W), bool)
    for dy in range(3):
        for dx in range(3):
            m1 |= m0p[:, dy:dy + H, dx:dx + W]
    m4 = np.zeros((B, 2 * H + 1, 2 * W + 1), bool)
    for dy in range(3):
        for dx in range(3):
            m4[:, dy:dy + 2 * H - 1:2, dx:dx + 2 * W - 1:2] |= m1

    w1c, w2c, w3c, wtc, w5c = _pack_weights(w1, w2, w3, wt, w5)
    biases = np.zeros((128, 5), f32)
    for i, bb in enumerate((b1, b2, b3, bt, b5)):
        biases[0:64, i] = bb
        biases[64:128, i] = bb

    in_maps = []
    for core in range(8):
        b, half = core // 2, core % 2
        # padded dense image: rows -8..263, cols -1..257
        Xp = np.zeros((272, 259, 32), f32)
        Xp[8:8 + H, 1:1 + W] = dense[b]
        xs = np.empty((128, 72, NCOL), BF)
        for rp in range(2):
            for cs in range(2):
                v = Xp[128 * half + rp:128 * half + rp + 144:2,
                       cs:cs + NCOL, :]
                xs[32 * cs + 64 * rp:32 * cs + 64 * rp + 32] = \
                    v.transpose(2, 0, 1).astype(BF)
        M1p = np.zeros((272, NCOL), f32)
        M1p[8:8 + H, 1:1 + W] = m1[b]
        m1d = np.empty((128, 70, NCOL), BF)
        for rp in range(2):
            v = M1p[128 * half + 2 + rp:128 * half + 2 + rp + 140:2, :]
            m1d[64 * rp:64 * rp + 64] = np.broadcast_to(
                v.astype(BF)[None], (64, 70, NCOL))
        M4p = np.zeros((520, P4), f32)
        M4p[0:513, 0:513] = m4[b]
        m4d = np.empty((128, 129, P4), BF)
        for rp in range(2):
            v = M4p[256 * half + rp:256 * half + rp + 258:2, :]
            m4d[64 * rp:64 * rp + 64] = np.broadcast_to(
                v.astype(BF)[None], (64, 129, P4))
        in_maps.append(dict(
            xs=np.ascontiguousarray(xs),
            m1d=np.ascontiguousarray(m1d),
            m4d=np.ascontiguousarray(m4d),
            w1c=w1c, w2c=w2c, w3c=w3c, wtc=wtc, w5c=w5c, biases=biases,
        ))
    return in_maps


def _build_program():
    nc = bacc.Bacc("TRN2", target_bir_lowering=False, debug=False,
                   enable_asserts=True, num_devices=8)

    xs_d = nc.dram_tensor("xs", [128, 72, NCOL], BF16,
                          kind="ExternalInput").ap()
    m1_d = nc.dram_tensor("m1d", [128, 70, NCOL], BF16,
                          kind="ExternalInput").ap()
    m4_d = nc.dram_tensor("m4d", [128, 129, P4], BF16,
                          kind="ExternalInput").ap()
    w1_d = nc.dram_tensor("w1c", [128, 6, 128], BF16,
                          kind="ExternalInput").ap()
    w2_d = nc.dram_tensor("w2c", [128, 9, 128], BF16,
                          kind="ExternalInput").ap()
    w3_d = nc.dram_tensor("w3c", [128, 9, 128], BF16,
                          kind="ExternalInput").ap()
    wt_d = nc.dram_tensor("wtc", [128, 3, 128], BF16,
                          kind="ExternalInput").ap()
    w5_d = nc.dram_tensor("w5c", [128, 6, 128], BF16,
                          kind="ExternalInput").ap()
    bias_d = nc.dram_tensor("biases", [128, 5], F32,
                            kind="ExternalInput").ap()
    out_d = nc.dram_tensor("out", [128, 128, 511], BF16,
                           kind="ExternalOutput").ap()

    with tile.TileContext(nc) as tc, ExitStack() as ctx:
        wp = ctx.enter_context(tc.tile_pool(name="wp", bufs=1))
        xp = ctx.enter_context(tc.tile_pool(name="xp", bufs=2))
        mp = ctx.enter_context(tc.tile_pool(name="mp", bufs=2))
        hp = ctx.enter_context(tc.tile_pool(name="hp", bufs=1))
        pp = ctx.enter_context(tc.tile_pool(name="pp", bufs=8, space="PSUM"))
        op = ctx.enter_context(tc.tile_pool(name="op", bufs=4))

        w1t = wp.tile([128, 6, 128], BF16, name="w1t")
        w2t = wp.tile([128, 9, 128], BF16, name="w2t")
        w3t = wp.tile([128, 9, 128], BF16, name="w3t")
        wtt = wp.tile([128, 3, 128], BF16, name="wtt")
        w5t = wp.tile([128, 6, 128], BF16, name="w5t")
        bt = wp.tile([128, 5], F32, name="bt")
        nc.sync.dma_start(w1t[:], w1_d[:])
        nc.sync.dma_start(bt[:], bias_d[:])
        nc.scalar.dma_start(w2t[:], w2_d[:])
        nc.scalar.dma_start(w3t[:], w3_d[:])
        nc.gpsimd.dma_start(wtt[:], wt_d[:])
        nc.gpsimd.dma_start(w5t[:], w5_d[:])

        def conv_layer(inp, wt_, deltas, nslots, bias_ap, m_ch, moff, h_out):
            nc.gpsimd.memset(h_out[:, :, 0:NCOL:NCOL - 1], 0)
            for t0 in range(0, nslots, 2):
                nt = min(2, nslots - t0)
                pc = pp.tile([128, 2, 256], F32, name="pc", tag="ps")
                ps = pc[:, 0:nt, :]
                for k, (dlt, d) in enumerate(deltas):
                    nc.tensor.matmul(
                        ps, wt_[:, k, :],
                        inp[:, t0 + 1 + dlt:t0 + 1 + dlt + nt, d:d + 256],
                        start=(k == 0), stop=(k == len(deltas) - 1))
                dst = h_out[:, t0:t0 + nt, 1:257]
                nc.scalar.activation(dst, ps, RELU, bias=bias_ap)
                nc.vector.tensor_mul(
                    dst, dst, m_ch[:, t0 + moff:t0 + moff + nt, 1:257])

        def load_chunk(c):
            x_ch = xp.tile([128, 16, NCOL], BF16, name="x_ch", tag="x")
            nc.sync.dma_start(x_ch[:], xs_d[:, 8 * c:8 * c + 16, :])
            m1_ch = mp.tile([128, 14, NCOL], BF16, name="m1_ch", tag="m1")
            nc.sync.dma_start(m1_ch[:], m1_d[:, 8 * c:8 * c + 14, :])
            m4_ch = mp.tile([128, 17, P4], BF16, name="m4_ch", tag="m4")
            nc.sync.dma_start(m4_ch[:], m4_d[:, 16 * c:16 * c + 17, :])
            return x_ch, m1_ch, m4_ch

        def emit_convT(h3, h3o, h4, m4_ch):
            for l0, nb in TBATCH:
                T, ii = (h3o, l0 // 2) if l0 % 2 == 0 else (h3, (l0 + 1) // 2)
                for q in range(nb):
                    pe = pp.tile([128, 257], F32, name="pe", tag="ps")
                    nc.tensor.matmul(pe[:], wtt[:, 0, :],
                                     T[:, ii + q, 1:258],
                                     start=True, stop=False)
                    nc.tensor.matmul(pe[:], wtt[:, 1, :],
                                     T[:, ii + q, 0:257],
                                     start=False, stop=True)
                    de = h4[:, l0 + 2 * q, 0:513:2]
                    nc.scalar.activation(de, pe[:], RELU, bias=bt[:, 3:4])
                po = pp.tile([128, 2, 256], F32, name="po", tag="ps")
                nc.tensor.matmul(po[:, 0:nb, :], wtt[:, 2, :],
                                 T[:, ii:ii + nb, 1:257],
                                 start=True, stop=True)
                do = h4[:, l0:l0 + 2 * nb - 1:2, 1:512:2]
                nc.scalar.activation(do, po[:, 0:nb, :], RELU, bias=bt[:, 3:4])
            nc.vector.tensor_mul(h4[:, 0:8, 0:513], h4[:, 0:8, 0:513],
                                 m4_ch[:, 0:8, 0:513])
            nc.vector.tensor_mul(h4[:, 8:17, 0:513], h4[:, 8:17, 0:513],
                                 m4_ch[:, 8:17, 0:513])

        def emit_conv5(h4, c, r0s):
            for r0 in r0s:
                out_sb = op.tile([128, 2, 511], BF16, name="out_sb", tag="o")
                for q in range(2):
                    p5 = pp.tile([128, 512], F32, name="p5", tag="ps")
                    for k, (sg, d) in enumerate(SIGD):
                        nc.tensor.matmul(p5[:], w5t[:, k, :],
                                         h4[:, r0 + q + sg, d:d + 512],
                                         start=(k == 0), stop=(k == 5))
                    nc.scalar.activation(out_sb[:, q, :], p5[:, 0:511], RELU,
                                         bias=bt[:, 4:5])
                nc.gpsimd.dma_start(
                    out_d[:, 16 * c + r0:16 * c + r0 + 2, :], out_sb[:])

        prev = None  # (h3, h3o, h4, m4_ch, c)
        nxt = load_chunk(0)
        for it in range(NCH + 1):
            if it < NCH:
                x_ch, m1_ch, m4_ch = nxt
                if it + 1 < NCH:
                    nxt = load_chunk(it + 1)
                h1 = hp.tile([128, 14, NCOL], BF16, name="h1", tag="h1")
                h2 = hp.tile([128, 12, NCOL], BF16, name="h2", tag="h2")
                h3 = hp.tile([128, 10, NCOL], BF16, name="h3", tag="h3",
                             bufs=2)
                h3o = hp.tile([128, 9, NCOL], BF16, name="h3o", tag="h3o",
                              bufs=2)
                conv_layer(x_ch, w1t, DELTAS1, 14, bt[:, 0:1], m1_ch, 0, h1)
            if prev is not None:
                h3p, h3op, h4p, m4p_, cp = prev
                emit_convT(h3p, h3op, h4p, m4p_)
            if it < NCH:
                conv_layer(h1, w2t, DELTAS9, 12, bt[:, 1:2], m1_ch, 1, h2)
            if prev is not None:
                emit_conv5(h4p, cp, range(0, 8, 2))
            if it < NCH:
                conv_layer(h2, w3t, DELTAS9, 10, bt[:, 2:3], m1_ch, 2, h3)
                nc.vector.tensor_copy(h3o[0:64, :, :], h3[64:128, 0:9, :])
                nc.vector.tensor_copy(h3o[64:128, :, :], h3[0:64, 1:10, :])
            if prev is not None:
                emit_conv5(h4p, cp, range(8, 16, 2))
            if it < NCH:
                h4 = hp.tile([128, 17, P4], BF16, name="h4", tag="h4")
                prev = (h3, h3o, h4, m4_ch, it)
            else:
                prev = None

    nc.compile()
    return nc


def kernel(**inputs):
    features = np.asarray(inputs["features"], np.float32)
    coors = np.asarray(inputs["coors"], np.int32)
    args = [np.asarray(inputs[k], np.float32) for k in
            ("w1", "b1", "w2", "b2", "w3", "b3", "wt", "bt", "w5", "b5")]
    in_maps = _host_prep(features, coors, *args)
    if "nc" not in _CACHE:
        _CACHE["nc"] = _build_program()
    res = run_bass_kernel_spmd(_CACHE["nc"], in_maps,
                               core_ids=list(range(8)), trace=False)
    full = np.zeros((B, 511, 511, 64), np.float32)
    for core in range(8):
        b, half = core // 2, core % 2
        o = np.asarray(res.results[core]["out"]).astype(np.float32)
        rows = o.reshape(2, 64, 128, 511).transpose(2, 0, 3, 1) \
            .reshape(256, 511, 64)
        nrow = 256 if half == 0 else 255
        full[b, 256 * half:256 * half + nrow] = rows[:nrow]
    return full


# revision 14
# speedup vs baseline: 1.8797x; 1.0591x over previous
"""Trainium2 Bass kernel for the sparse-conv network (nn_ExampleNet).

Pipeline (per batch image): scatter 200k sparse voxel features into a dense
[256,256,32] grid, SparseConv(32->64) + 2x SubMConv(64) with an active-site
mask, SparseConvTranspose(64, stride 2), dense 3x3 VALID conv -> [511,511,64].

Strategy: 8-way SPMD (4 batches x 2 row-halves), row-parity-packed layout:
every on-chip tensor stores row pairs across the 128 partitions
(partitions 0:64 = even row channels, 64:128 = odd row), so every matmul
uses all 128 PE output lanes and every activation/mask-multiply runs at
128-partition throughput. Everything stored in bf16 (halves SBUF + DMA +
doubles DVE rate); PSUM accumulation stays fp32.
"""
from contextlib import ExitStack

import numpy as np
import ml_dtypes

import concourse.bacc as bacc
import concourse.mybir as mybir
import concourse.tile as tile
from concourse.bass_utils import run_bass_kernel_spmd

F32 = mybir.dt.float32
F32R = mybir.dt.float32r
BF16 = mybir.dt.bfloat16
RELU = mybir.ActivationFunctionType.Relu
BF = ml_dtypes.bfloat16

B, H, W = 4, 256, 256
NCOL = 258      # padded col count for x/h1-3 slabs
P4 = 516        # h4/m4 col pitch
CH = 64         # output rows per chunk
NCH = 256 // CH
SADV = CH // 4  # x/h slot advance per chunk
NUP = CH // 2 + 1           # h4 up-pairs per chunk
S3, S2, S1, SX = CH // 4 + 2, CH // 4 + 4, CH // 4 + 6, CH // 4 + 8

DELTAS1 = [(-1, 0), (-1, 2), (0, 0), (0, 2), (1, 0), (1, 2)]
DELTAS9 = [(dlt, d) for dlt in (-1, 0, 1) for d in range(3)]
SIGD = [(sg, d) for sg in (0, 1) for d in range(3)]
# convT up-pair batches: (first local up-pair, count); even-l use the
# odd-aligned h3 copy, odd-l the standard one
TBATCH = ([(l0, min(2, (NUP - l0 + 1) // 2)) for l0 in range(0, NUP, 4)]
          + [(l0, min(2, (NUP - l0 + 1) // 2)) for l0 in range(1, NUP, 4)])

_CACHE = {}


def _pack_weights(w1, w2, w3, wt, w5):
    f32 = np.float32
    w1c = np.zeros((128, 6, 128), f32)
    for k, (dlt, d) in enumerate(DELTAS1):
        for rp in range(2):
            for cs in range(2):
                for op in range(2):
                    dy = 2 * dlt + rp - op + 1
                    dx = d + cs
                    if 0 <= dy < 3 and 0 <= dx < 3:
                        w1c[32 * cs + 64 * rp:32 * cs + 64 * rp + 32,
                            k, 64 * op:64 * op + 64] = w1[dy, dx]

    def pack9(wm):
        out = np.zeros((128, 9, 128), f32)
        for k, (dlt, d) in enumerate(DELTAS9):
            for rp in range(2):
                for op in range(2):
                    dy = 2 * dlt + rp - op + 1
                    if 0 <= dy < 3:
                        out[64 * rp:64 * rp + 64, k,
                            64 * op:64 * op + 64] = wm[dy, d]
        return out

    wte = wt[::-1, ::-1]  # jax conv_transpose applies the flipped kernel
    wtc = np.zeros((128, 3, 128), f32)
    for k, dx in enumerate((0, 2, 1)):
        wtc[0:64, k, 0:64] = wte[2, dx]
        wtc[64:128, k, 0:64] = wte[0, dx]
        wtc[64:128, k, 64:128] = wte[1, dx]
    w5c = np.zeros((128, 6, 128), f32)
    for k, (sg, d) in enumerate(SIGD):
        for rp in range(2):
            for op in range(2):
                dy = 2 * sg + rp - op
                if 0 <= dy < 3:
                    w5c[64 * rp:64 * rp + 64, k,
                        64 * op:64 * op + 64] = w5[dy, d]
    return tuple(w.astype(BF) for w in
                 (w1c, pack9(w2), pack9(w3), wtc, w5c))


def _host_prep(features, coors, w1, b1, w2, b2, w3, b3, wt, bt, w5, b5):
    f32 = np.float32
    bi, yi, xi = coors[:, 0], coors[:, 1], coors[:, 2]
    flat = (bi.astype(np.int64) * H + yi) * W + xi
    dense = np.zeros((B * H * W, 32), f32)
    for c in range(32):
        dense[:, c] = np.bincount(flat, weights=features[:, c],
                                  minlength=B * H * W)
    dense = dense.reshape(B, H, W, 32)
    occ = np.bincount(flat, minlength=B * H * W).reshape(B, H, W) > 0
    m0p = np.zeros((B, H + 2, W + 2), bool)
    m0p[:, 1:-1, 1:-1] = occ
    m1 = np.zeros((B, H, W), bool)
    for dy in range(3):
        for dx in range(3):
            m1 |= m0p[:, dy:dy + H, dx:dx + W]
    m4 = np.zeros((B, 2 * H + 1, 2 * W + 1), bool)
    for dy in range(3):
        for dx in range(3):
            m4[:, dy:dy + 2 * H - 1:2, dx:dx + 2 * W - 1:2] |= m1

    w1c, w2c, w3c, wtc, w5c = _pack_weights(w1, w2, w3, wt, w5)
    biases = np.zeros((128, 5), f32)
    for i, bb in enumerate((b1, b2, b3, bt, b5)):
        biases[0:64, i] = bb
        biases[64:128, i] = bb

    in_maps = []
    for core in range(8):
        b, half = core // 2, core % 2
        # padded dense image: rows -8..263, cols -1..257
        Xp = np.zeros((272, 259, 32), f32)
        Xp[8:8 + H, 1:1 + W] = dense[b]
        xs = np.empty((128, 72, NCOL), BF)
        for rp in range(2):
            for cs in range(2):
                v = Xp[128 * half + rp:128 * half + rp + 144:2,
                       cs:cs + NCOL, :]
                xs[32 * cs + 64 * rp:32 * cs + 64 * rp + 32] = \
                    v.transpose(2, 0, 1).astype(BF)
        M1p = np.zeros((272, NCOL), f32)
        M1p[8:8 + H, 1:1 + W] = m1[b]
        m1d = np.empty((128, 70, NCOL), BF)
        for rp in range(2):
            v = M1p[128 * half + 2 + rp:128 * half + 2 + rp + 140:2, :]
            m1d[64 * rp:64 * rp + 64] = np.broadcast_to(
                v.astype(BF)[None], (64, 70, NCOL))
        M4p = np.zeros((520, P4), f32)
        M4p[0:513, 0:513] = m4[b]
        m4d = np.empty((128, 129, P4), BF)
        for rp in range(2):
            v = M4p[256 * half + rp:256 * half + rp + 258:2, :]
            m4d[64 * rp:64 * rp + 64] = np.broadcast_to(
                v.astype(BF)[None], (64, 129, P4))
        in_maps.append(dict(
            xs=np.ascontiguousarray(xs),
            m1d=np.ascontiguousarray(m1d),
            m4d=np.ascontiguousarray(m4d),
            w1c=w1c, w2c=w2c, w3c=w3c, wtc=wtc, w5c=w5c, biases=biases,
        ))
    return in_maps


def _build_program():
    nc = bacc.Bacc("TRN2", target_bir_lowering=False, debug=False,
                   enable_asserts=True, num_devices=8)

    xs_d = nc.dram_tensor("xs", [128, 72, NCOL], BF16,
                          kind="ExternalInput").ap()
    m1_d = nc.dram_tensor("m1d", [128, 70, NCOL], BF16,
                          kind="ExternalInput").ap()
    m4_d = nc.dram_tensor("m4d", [128, 129, P4], BF16,
                          kind="ExternalInput").ap()
    w1_d = nc.dram_tensor("w1c", [128, 6, 128], BF16,
                          kind="ExternalInput").ap()
    w2_d = nc.dram_tensor("w2c", [128, 9, 128], BF16,
                          kind="ExternalInput").ap()
    w3_d = nc.dram_tensor("w3c", [128, 9, 128], BF16,
                          kind="ExternalInput").ap()
    wt_d = nc.dram_tensor("wtc", [128, 3, 128], BF16,
                          kind="ExternalInput").ap()
    w5_d = nc.dram_tensor("w5c", [128, 6, 128], BF16,
                          kind="ExternalInput").ap()
    bias_d = nc.dram_tensor("biases", [128, 5], F32,
                            kind="ExternalInput").ap()
    out_d = nc.dram_tensor("out", [128, 128, 511], BF16,
                           kind="ExternalOutput").ap()

    with tile.TileContext(nc) as tc, ExitStack() as ctx:
        wp = ctx.enter_context(tc.tile_pool(name="wp", bufs=1))
        xp = ctx.enter_context(tc.tile_pool(name="xp", bufs=2))
        mp = ctx.enter_context(tc.tile_pool(name="mp", bufs=2))
        hp = ctx.enter_context(tc.tile_pool(name="hp", bufs=1))
        pp = ctx.enter_context(tc.tile_pool(name="pp", bufs=8, space="PSUM"))
        op = ctx.enter_context(tc.tile_pool(name="op", bufs=4))

        w1t = wp.tile([128, 6, 128], BF16, name="w1t")
        w2t = wp.tile([128, 9, 128], BF16, name="w2t")
        w3t = wp.tile([128, 9, 128], BF16, name="w3t")
        wtt = wp.tile([128, 3, 128], BF16, name="wtt")
        w5t = wp.tile([128, 6, 128], BF16, name="w5t")
        bt = wp.tile([128, 5], F32, name="bt")
        nc.sync.dma_start(w1t[:], w1_d[:])
        nc.sync.dma_start(bt[:], bias_d[:])
        nc.scalar.dma_start(w2t[:], w2_d[:])
        nc.scalar.dma_start(w3t[:], w3_d[:])
        nc.gpsimd.dma_start(wtt[:], wt_d[:])
        nc.gpsimd.dma_start(w5t[:], w5_d[:])

        def conv_layer(inp, wt_, deltas, nslots, bias_ap, m_ch, moff, h_out):
            nc.gpsimd.memset(h_out[:, :, 0:NCOL:NCOL - 1], 0)
            for t0 in range(0, nslots, 2):
                nt = min(2, nslots - t0)
                pc = pp.tile([128, 2, 256], F32, name="pc", tag="ps")
                ps = pc[:, 0:nt, :]
                for k, (dlt, d) in enumerate(deltas):
                    nc.tensor.matmul(
                        ps, wt_[:, k, :],
                        inp[:, t0 + 1 + dlt:t0 + 1 + dlt + nt, d:d + 256],
                        start=(k == 0), stop=(k == len(deltas) - 1))
                dst = h_out[:, t0:t0 + nt, 1:257]
                nc.scalar.activation(dst, ps, RELU, bias=bias_ap)
                nc.vector.tensor_mul(
                    dst, dst, m_ch[:, t0 + moff:t0 + moff + nt, 1:257])

        def load_chunk(c):
            x_ch = xp.tile([128, SX, NCOL], BF16, name="x_ch", tag="x")
            nc.sync.dma_start(x_ch[:], xs_d[:, SADV * c:SADV * c + SX, :])
            m1_ch = mp.tile([128, S1, NCOL], BF16, name="m1_ch", tag="m1")
            nc.sync.dma_start(m1_ch[:], m1_d[:, SADV * c:SADV * c + S1, :])
            m4_ch = mp.tile([128, NUP, P4], BF16, name="m4_ch", tag="m4",
                            bufs=1)
            nc.scalar.dma_start(m4_ch[:], m4_d[:, (CH // 2) * c:
                                               (CH // 2) * c + NUP, :])
            return x_ch, m1_ch, m4_ch

        def emit_convT(h3, h3o, h4, m4_ch):
            for l0, nb in TBATCH:
                T, ii = (h3o, l0 // 2) if l0 % 2 == 0 else (h3, (l0 + 1) // 2)
                for q in range(nb):
                    pe = pp.tile([128, 257], F32, name="pe", tag="ps")
                    nc.tensor.matmul(pe[:], wtt[:, 0, :],
                                     T[:, ii + q, 1:258],
                                     start=True, stop=False)
                    nc.tensor.matmul(pe[:], wtt[:, 1, :],
                                     T[:, ii + q, 0:257],
                                     start=False, stop=True)
                    de = h4[:, l0 + 2 * q, 0:513:2]
                    nc.scalar.activation(de, pe[:], RELU, bias=bt[:, 3:4])
                po = pp.tile([128, 2, 256], F32, name="po", tag="ps")
                nc.tensor.matmul(po[:, 0:nb, :], wtt[:, 2, :],
                                 T[:, ii:ii + nb, 1:257],
                                 start=True, stop=True)
                do = h4[:, l0:l0 + 2 * nb - 1:2, 1:512:2]
                nc.scalar.activation(do, po[:, 0:nb, :], RELU, bias=bt[:, 3:4])
            hm = NUP // 2
            nc.vector.tensor_mul(h4[:, 0:hm, 0:513], h4[:, 0:hm, 0:513],
                                 m4_ch[:, 0:hm, 0:513])
            nc.vector.tensor_mul(h4[:, hm:NUP, 0:513], h4[:, hm:NUP, 0:513],
                                 m4_ch[:, hm:NUP, 0:513])

        def emit_conv5(h4, c, r0s):
            for r0 in r0s:
                out_sb = op.tile([128, 2, 511], BF16, name="out_sb", tag="o")
                for q in range(2):
                    p5 = pp.tile([128, 512], F32, name="p5", tag="ps")
                    for k, (sg, d) in enumerate(SIGD):
                        nc.tensor.matmul(p5[:], w5t[:, k, :],
                                         h4[:, r0 + q + sg, d:d + 512],
                                         start=(k == 0), stop=(k == 5))
                    nc.scalar.activation(out_sb[:, q, :], p5[:, 0:511], RELU,
                                         bias=bt[:, 4:5])
                nc.gpsimd.dma_start(
                    out_d[:, (CH // 2) * c + r0:(CH // 2) * c + r0 + 2, :],
                    out_sb[:])

        prev = None  # (h3, h3o, h4, m4_ch, c)
        nxt = load_chunk(0)
        for it in range(NCH + 1):
            if it < NCH:
                x_ch, m1_ch, m4_ch = nxt
                if it + 1 < NCH:
                    nxt = load_chunk(it + 1)
                h1 = hp.tile([128, S1, NCOL], BF16, name="h1", tag="h1")
                h2 = hp.tile([128, S2, NCOL], BF16, name="h2", tag="h2")
                h3 = hp.tile([128, S3, NCOL], BF16, name="h3", tag="h3",
                             bufs=2)
                h3o = hp.tile([128, S3 - 1, NCOL], BF16, name="h3o",
                              tag="h3o", bufs=2)
                conv_layer(x_ch, w1t, DELTAS1, S1, bt[:, 0:1], m1_ch, 0, h1)
            if prev is not None:
                h3p, h3op, h4p, m4p_, cp = prev
                emit_convT(h3p, h3op, h4p, m4p_)
            if it < NCH:
                conv_layer(h1, w2t, DELTAS9, S2, bt[:, 1:2], m1_ch, 1, h2)
            if prev is not None:
                emit_conv5(h4p, cp, range(0, CH // 4, 2))
            if it < NCH:
                conv_layer(h2, w3t, DELTAS9, S3, bt[:, 2:3], m1_ch, 2, h3)
                nc.vector.tensor_copy(h3o[0:64, :, :],
                                      h3[64:128, 0:S3 - 1, :])
                nc.vector.tensor_copy(h3o[64:128, :, :], h3[0:64, 1:S3, :])
            if prev is not None:
                emit_conv5(h4p, cp, range(CH // 4, CH // 2, 2))
            if it < NCH:
                h4 = hp.tile([128, NUP, P4], BF16, name="h4", tag="h4")
                prev = (h3, h3o, h4, m4_ch, it)
            else:
                prev = None

    nc.compile()
    return nc


def kernel(**inputs):
    features = np.asarray(inputs["features"], np.float32)
    coors = np.asarray(inputs["coors"], np.int32)
    args = [np.asarray(inputs[k], np.float32) for k in
            ("w1", "b1", "w2", "b2", "w3", "b3", "wt", "bt", "w5", "b5")]
    in_maps = _host_prep(features, coors, *args)
    if "nc" not in _CACHE:
        _CACHE["nc"] = _build_program()
    res = run_bass_kernel_spmd(_CACHE["nc"], in_maps,
                               core_ids=list(range(8)), trace=False)
    full = np.zeros((B, 511, 511, 64), np.float32)
    for core in range(8):
        b, half = core // 2, core % 2
        o = np.asarray(res.results[core]["out"]).astype(np.float32)
        rows = o.reshape(2, 64, 128, 511).transpose(2, 0, 3, 1) \
            .reshape(256, 511, 64)
        nrow = 256 if half == 0 else 255
        full[b, 256 * half:256 * half + nrow] = rows[:nrow]
    return full


# revision 17
# speedup vs baseline: 1.9593x; 1.0423x over previous
"""Trainium2 Bass kernel for the sparse-conv network (nn_ExampleNet).

Pipeline (per batch image): scatter 200k sparse voxel features into a dense
[256,256,32] grid, SparseConv(32->64) + 2x SubMConv(64) with an active-site
mask, SparseConvTranspose(64, stride 2), dense 3x3 VALID conv -> [511,511,64].

Strategy: 8-way SPMD (4 batches x 2 row-halves), row-parity-packed layout:
every on-chip tensor stores row pairs across the 128 partitions
(partitions 0:64 = even row channels, 64:128 = odd row), so every matmul
uses all 128 PE output lanes and every activation/mask-multiply runs at
128-partition throughput. Everything stored in bf16 (halves SBUF + DMA +
doubles DVE rate); PSUM accumulation stays fp32.
"""
from contextlib import ExitStack

import numpy as np
import ml_dtypes

import concourse.bacc as bacc
import concourse.mybir as mybir
import concourse.tile as tile
from concourse.bass_utils import run_bass_kernel_spmd

F32 = mybir.dt.float32
F32R = mybir.dt.float32r
BF16 = mybir.dt.bfloat16
RELU = mybir.ActivationFunctionType.Relu
BF = ml_dtypes.bfloat16

B, H, W = 4, 256, 256
NCOL = 258      # padded col count for x/h1-3 slabs
P4 = 516        # h4/m4 col pitch
CH = 64         # output rows per chunk
NCH = 256 // CH
SADV = CH // 4  # x/h slot advance per chunk
NUP = CH // 2 + 1           # h4 up-pairs per chunk
S3, S2, S1, SX = CH // 4 + 2, CH // 4 + 4, CH // 4 + 6, CH // 4 + 8

DELTAS1 = [(-1, 0), (-1, 2), (0, 0), (0, 2), (1, 0), (1, 2)]
DELTAS9 = [(dlt, d) for dlt in (-1, 0, 1) for d in range(3)]
SIGD = [(sg, d) for sg in (0, 1) for d in range(3)]
# convT up-pair batches: (first local up-pair, count); even-l use the
# odd-aligned h3 copy, odd-l the standard one
TBATCH = ([(l0, min(2, (NUP - l0 + 1) // 2)) for l0 in range(0, NUP, 4)]
          + [(l0, min(2, (NUP - l0 + 1) // 2)) for l0 in range(1, NUP, 4)])

_CACHE = {}


def _pack_weights(w1, w2, w3, wt, w5):
    f32 = np.float32
    w1c = np.zeros((128, 6, 128), f32)
    for k, (dlt, d) in enumerate(DELTAS1):
        for rp in range(2):
            for cs in range(2):
                for op in range(2):
                    dy = 2 * dlt + rp - op + 1
                    dx = d + cs
                    if 0 <= dy < 3 and 0 <= dx < 3:
                        w1c[32 * cs + 64 * rp:32 * cs + 64 * rp + 32,
                            k, 64 * op:64 * op + 64] = w1[dy, dx]

    def pack9(wm):
        out = np.zeros((128, 9, 128), f32)
        for k, (dlt, d) in enumerate(DELTAS9):
            for rp in range(2):
                for op in range(2):
                    dy = 2 * dlt + rp - op + 1
                    if 0 <= dy < 3:
                        out[64 * rp:64 * rp + 64, k,
                            64 * op:64 * op + 64] = wm[dy, d]
        return out

    wte = wt[::-1, ::-1]  # jax conv_transpose applies the flipped kernel
    wtc = np.zeros((128, 3, 128), f32)
    for k, dx in enumerate((0, 2, 1)):
        wtc[0:64, k, 0:64] = wte[2, dx]
        wtc[64:128, k, 0:64] = wte[0, dx]
        wtc[64:128, k, 64:128] = wte[1, dx]
    w5c = np.zeros((128, 6, 128), f32)
    for k, (sg, d) in enumerate(SIGD):
        for rp in range(2):
            for op in range(2):
                dy = 2 * sg + rp - op
                if 0 <= dy < 3:
                    w5c[64 * rp:64 * rp + 64, k,
                        64 * op:64 * op + 64] = w5[dy, d]
    return tuple(w.astype(BF) for w in
                 (w1c, pack9(w2), pack9(w3), wtc, w5c))


def _host_prep(features, coors, w1, b1, w2, b2, w3, b3, wt, bt, w5, b5):
    f32 = np.float32
    bi, yi, xi = coors[:, 0], coors[:, 1], coors[:, 2]
    flat = (bi.astype(np.int64) * H + yi) * W + xi
    dense = np.zeros((B * H * W, 32), f32)
    for c in range(32):
        dense[:, c] = np.bincount(flat, weights=features[:, c],
                                  minlength=B * H * W)
    dense = dense.reshape(B, H, W, 32)
    occ = np.bincount(flat, minlength=B * H * W).reshape(B, H, W) > 0
    m0p = np.zeros((B, H + 2, W + 2), bool)
    m0p[:, 1:-1, 1:-1] = occ
    m1 = np.zeros((B, H, W), bool)
    for dy in range(3):
        for dx in range(3):
            m1 |= m0p[:, dy:dy + H, dx:dx + W]
    m4 = np.zeros((B, 2 * H + 1, 2 * W + 1), bool)
    for dy in range(3):
        for dx in range(3):
            m4[:, dy:dy + 2 * H - 1:2, dx:dx + 2 * W - 1:2] |= m1

    w1c, w2c, w3c, wtc, w5c = _pack_weights(w1, w2, w3, wt, w5)
    biases = np.zeros((128, 5), f32)
    for i, bb in enumerate((b1, b2, b3, bt, b5)):
        biases[0:64, i] = bb
        biases[64:128, i] = bb

    in_maps = []
    for core in range(8):
        b, half = core // 2, core % 2
        # padded dense image: rows -8..263, cols -1..257
        Xp = np.zeros((272, 259, 32), f32)
        Xp[8:8 + H, 1:1 + W] = dense[b]
        xs = np.empty((128, 72, NCOL), BF)
        for rp in range(2):
            for cs in range(2):
                v = Xp[128 * half + rp:128 * half + rp + 144:2,
                       cs:cs + NCOL, :]
                xs[32 * cs + 64 * rp:32 * cs + 64 * rp + 32] = \
                    v.transpose(2, 0, 1).astype(BF)
        M1p = np.zeros((272, NCOL), f32)
        M1p[8:8 + H, 1:1 + W] = m1[b]
        m1d = np.empty((128, 70, NCOL), BF)
        for rp in range(2):
            v = M1p[128 * half + 2 + rp:128 * half + 2 + rp + 140:2, :]
            m1d[64 * rp:64 * rp + 64] = np.broadcast_to(
                v.astype(BF)[None], (64, 70, NCOL))
        M4p = np.zeros((520, P4), f32)
        M4p[0:513, 0:513] = m4[b]
        m4d = np.empty((128, 129, P4), BF)
        for rp in range(2):
            v = M4p[256 * half + rp:256 * half + rp + 258:2, :]
            m4d[64 * rp:64 * rp + 64] = np.broadcast_to(
                v.astype(BF)[None], (64, 129, P4))
        in_maps.append(dict(
            xs=np.ascontiguousarray(xs),
            m1d=np.ascontiguousarray(m1d),
            m4d=np.ascontiguousarray(m4d),
            w1c=w1c, w2c=w2c, w3c=w3c, wtc=wtc, w5c=w5c, biases=biases,
        ))
    return in_maps


def _build_program():
    nc = bacc.Bacc("TRN2", target_bir_lowering=False, debug=False,
                   enable_asserts=True, num_devices=8)

    xs_d = nc.dram_tensor("xs", [128, 72, NCOL], BF16,
                          kind="ExternalInput").ap()
    m1_d = nc.dram_tensor("m1d", [128, 70, NCOL], BF16,
                          kind="ExternalInput").ap()
    m4_d = nc.dram_tensor("m4d", [128, 129, P4], BF16,
                          kind="ExternalInput").ap()
    w1_d = nc.dram_tensor("w1c", [128, 6, 128], BF16,
                          kind="ExternalInput").ap()
    w2_d = nc.dram_tensor("w2c", [128, 9, 128], BF16,
                          kind="ExternalInput").ap()
    w3_d = nc.dram_tensor("w3c", [128, 9, 128], BF16,
                          kind="ExternalInput").ap()
    wt_d = nc.dram_tensor("wtc", [128, 3, 128], BF16,
                          kind="ExternalInput").ap()
    w5_d = nc.dram_tensor("w5c", [128, 6, 128], BF16,
                          kind="ExternalInput").ap()
    bias_d = nc.dram_tensor("biases", [128, 5], F32,
                            kind="ExternalInput").ap()
    out_d = nc.dram_tensor("out", [128, 128, 511], BF16,
                           kind="ExternalOutput").ap()

    with tile.TileContext(nc) as tc, ExitStack() as ctx:
        wp = ctx.enter_context(tc.tile_pool(name="wp", bufs=1))
        xp = ctx.enter_context(tc.tile_pool(name="xp", bufs=2))
        mp = ctx.enter_context(tc.tile_pool(name="mp", bufs=2))
        hp = ctx.enter_context(tc.tile_pool(name="hp", bufs=1))
        pp = ctx.enter_context(tc.tile_pool(name="pp", bufs=4, space="PSUM"))
        op = ctx.enter_context(tc.tile_pool(name="op", bufs=4))

        h1 = wp.tile([128, S1, NCOL], BF16, name="h1buf")
        h2 = wp.tile([128, S2, NCOL], BF16, name="h2buf")
        w1t = wp.tile([128, 6, 128], BF16, name="w1t")
        w2t = wp.tile([128, 9, 128], BF16, name="w2t")
        w3t = wp.tile([128, 9, 128], BF16, name="w3t")
        wtt = wp.tile([128, 3, 128], BF16, name="wtt")
        w5t = wp.tile([128, 6, 128], BF16, name="w5t")
        bt = wp.tile([128, 5], F32, name="bt")
        nc.sync.dma_start(w1t[:], w1_d[:])
        nc.sync.dma_start(bt[:], bias_d[:])
        nc.scalar.dma_start(w2t[:], w2_d[:])
        nc.scalar.dma_start(w3t[:], w3_d[:])
        nc.gpsimd.dma_start(wtt[:], wt_d[:])
        nc.gpsimd.dma_start(w5t[:], w5_d[:])

        def conv_layer(inp, wt_, deltas, s_lo, nslots, bias_ap, m_ch, moff,
                       h_out):
            nc.gpsimd.memset(h_out[:, :, 0:NCOL:NCOL - 1], 0)
            for t0 in range(s_lo, nslots, 2):
                nt = min(2, nslots - t0)
                pc = pp.tile([128, 2, 256], F32, name="pc", tag="ps")
                ps = pc[:, 0:nt, :]
                for k, (dlt, d) in enumerate(deltas):
                    nc.tensor.matmul(
                        ps, wt_[:, k, :],
                        inp[:, t0 + 1 + dlt:t0 + 1 + dlt + nt, d:d + 256],
                        start=(k == 0), stop=(k == len(deltas) - 1))
                dst = h_out[:, t0:t0 + nt, 1:257]
                nc.scalar.activation(dst, ps, RELU, bias=bias_ap)
                nc.vector.tensor_mul(
                    dst, dst, m_ch[:, t0 + moff:t0 + moff + nt, 1:257])

        def load_chunk(c):
            x_ch = xp.tile([128, SX, NCOL], BF16, name="x_ch", tag="x")
            hx = SX // 2
            nc.sync.dma_start(x_ch[:, 0:hx, :],
                              xs_d[:, SADV * c:SADV * c + hx, :])
            nc.sync.dma_start(x_ch[:, hx:SX, :],
                              xs_d[:, SADV * c + hx:SADV * c + SX, :])
            m1_ch = mp.tile([128, S1, NCOL], BF16, name="m1_ch", tag="m1")
            nc.sync.dma_start(m1_ch[:], m1_d[:, SADV * c:SADV * c + S1, :])
            m4_ch = mp.tile([128, NUP, P4], BF16, name="m4_ch", tag="m4",
                            bufs=1)
            nc.scalar.dma_start(m4_ch[:], m4_d[:, (CH // 2) * c:
                                               (CH // 2) * c + NUP, :])
            return x_ch, m1_ch, m4_ch

        def emit_convT(h3, h3o, h4, m4_ch):
            for l0, nb in TBATCH:
                T, ii = (h3o, l0 // 2) if l0 % 2 == 0 else (h3, (l0 + 1) // 2)
                pe = pp.tile([128, 2, 512], F32, name="pe", tag="ps")
                for q in range(nb):
                    nc.tensor.matmul(pe[:, q, 0:257], wtt[:, 0, :],
                                     T[:, ii + q, 1:258],
                                     start=True, stop=False)
                    nc.tensor.matmul(pe[:, q, 0:257], wtt[:, 1, :],
                                     T[:, ii + q, 0:257],
                                     start=False, stop=True)
                de = h4[:, l0:l0 + 2 * nb - 1:2, 0:513:2]
                nc.scalar.activation(de, pe[:, 0:nb, 0:257], RELU,
                                     bias=bt[:, 3:4])
                po = pp.tile([128, 2, 256], F32, name="po", tag="ps")
                nc.tensor.matmul(po[:, 0:nb, :], wtt[:, 2, :],
                                 T[:, ii:ii + nb, 1:257],
                                 start=True, stop=True)
                do = h4[:, l0:l0 + 2 * nb - 1:2, 1:512:2]
                nc.scalar.activation(do, po[:, 0:nb, :], RELU, bias=bt[:, 3:4])
            for p0 in range(0, NUP, (NUP + 3) // 4):
                p1 = min(NUP, p0 + (NUP + 3) // 4)
                nc.vector.tensor_mul(h4[:, p0:p1, 0:513], h4[:, p0:p1, 0:513],
                                     m4_ch[:, p0:p1, 0:513])

        def emit_conv5(h4, c, r0s):
            for r0 in r0s:
                out_sb = op.tile([128, 2, 511], BF16, name="out_sb", tag="o")
                p5 = pp.tile([128, 2, 512], F32, name="p5", tag="ps")
                for q in range(2):
                    for k, (sg, d) in enumerate(SIGD):
                        nc.tensor.matmul(p5[:, q, :], w5t[:, k, :],
                                         h4[:, r0 + q + sg, d:d + 512],
                                         start=(k == 0), stop=(k == 5))
                nc.scalar.activation(out_sb[:], p5[:, :, 0:511], RELU,
                                     bias=bt[:, 4:5])
                nc.gpsimd.dma_start(
                    out_d[:, (CH // 2) * c + r0:(CH // 2) * c + r0 + 2, :],
                    out_sb[:])

        prev = None  # (h3, h3o, h4, m4_ch, c)
        h3prev = None
        nxt = load_chunk(0)
        for it in range(NCH + 1):
            if it < NCH:
                x_ch, m1_ch, m4_ch = nxt
                if it + 1 < NCH:
                    nxt = load_chunk(it + 1)
                h3 = hp.tile([128, S3, NCOL], BF16, name="h3", tag="h3",
                             bufs=2)
                h3o = hp.tile([128, S3 - 1, NCOL], BF16, name="h3o",
                              tag="h3o", bufs=2)
                # carry the exact boundary slots from the previous chunk
                # instead of recomputing them (h1/h2 are persistent tiles;
                # in-place copies between disjoint slot ranges)
                ov1, ov2, ov3 = S1 - SADV, S2 - SADV, S3 - SADV
                if it > 0:
                    nc.vector.tensor_copy(h1[:, 0:ov1, :],
                                          h1[:, SADV:S1, :])
                    nc.vector.tensor_copy(h2[:, 0:ov2, :],
                                          h2[:, SADV:S2, :])
                    nc.vector.tensor_copy(h3[:, 0:ov3, :],
                                          h3prev[:, SADV:S3, :])
                s1, s2, s3 = (ov1, ov2, ov3) if it > 0 else (0, 0, 0)
                conv_layer(x_ch, w1t, DELTAS1, s1, S1, bt[:, 0:1], m1_ch, 0,
                           h1)
            if prev is not None:
                h3p, h3op, h4p, m4p_, cp = prev
                emit_convT(h3p, h3op, h4p, m4p_)
            if it < NCH:
                conv_layer(h1, w2t, DELTAS9, s2, S2, bt[:, 1:2], m1_ch, 1, h2)
            if prev is not None:
                emit_conv5(h4p, cp, range(0, CH // 4, 2))
            if it < NCH:
                conv_layer(h2, w3t, DELTAS9, s3, S3, bt[:, 2:3], m1_ch, 2, h3)
                nc.vector.tensor_copy(h3o[0:64, :, :],
                                      h3[64:128, 0:S3 - 1, :])
                nc.vector.tensor_copy(h3o[64:128, :, :], h3[0:64, 1:S3, :])
            if prev is not None:
                emit_conv5(h4p, cp, range(CH // 4, CH // 2, 2))
            if it < NCH:
                h4 = hp.tile([128, NUP, P4], BF16, name="h4", tag="h4")
                prev = (h3, h3o, h4, m4_ch, it)
                h3prev = h3
            else:
                prev = None

    nc.compile()
    return nc


def kernel(**inputs):
    features = np.asarray(inputs["features"], np.float32)
    coors = np.asarray(inputs["coors"], np.int32)
    args = [np.asarray(inputs[k], np.float32) for k in
            ("w1", "b1", "w2", "b2", "w3", "b3", "wt", "bt", "w5", "b5")]
    in_maps = _host_prep(features, coors, *args)
    if "nc" not in _CACHE:
        _CACHE["nc"] = _build_program()
    res = run_bass_kernel_spmd(_CACHE["nc"], in_maps,
                               core_ids=list(range(8)), trace=False)
    full = np.zeros((B, 511, 511, 64), np.float32)
    for core in range(8):
        b, half = core // 2, core % 2
        o = np.asarray(res.results[core]["out"]).astype(np.float32)
        rows = o.reshape(2, 64, 128, 511).transpose(2, 0, 3, 1) \
            .reshape(256, 511, 64)
        nrow = 256 if half == 0 else 255
        full[b, 256 * half:256 * half + nrow] = rows[:nrow]
    return full


# revision 19
# speedup vs baseline: 1.9910x; 1.0161x over previous
"""Trainium2 Bass kernel for the sparse-conv network (nn_ExampleNet).

Pipeline (per batch image): scatter 200k sparse voxel features into a dense
[256,256,32] grid, SparseConv(32->64) + 2x SubMConv(64) with an active-site
mask, SparseConvTranspose(64, stride 2), dense 3x3 VALID conv -> [511,511,64].

Strategy: 8-way SPMD (4 batches x 2 row-halves), row-parity-packed layout:
every on-chip tensor stores row pairs across the 128 partitions
(partitions 0:64 = even row channels, 64:128 = odd row), so every matmul
uses all 128 PE output lanes and every activation/mask-multiply runs at
128-partition throughput. Everything stored in bf16 (halves SBUF + DMA +
doubles DVE rate); PSUM accumulation stays fp32.
"""
from contextlib import ExitStack

import numpy as np
import ml_dtypes

import concourse.bacc as bacc
import concourse.mybir as mybir
import concourse.tile as tile
from concourse.bass_utils import run_bass_kernel_spmd

F32 = mybir.dt.float32
F32R = mybir.dt.float32r
BF16 = mybir.dt.bfloat16
RELU = mybir.ActivationFunctionType.Relu
BF = ml_dtypes.bfloat16

B, H, W = 4, 256, 256
NCOL = 258      # padded col count for x/h1-3 slabs
P4 = 516        # h4/m4 col pitch
CH = 64         # output rows per chunk
NCH = 256 // CH
SADV = CH // 4  # x/h slot advance per chunk
NUP = CH // 2 + 1           # h4 up-pairs per chunk
S3, S2, S1, SX = CH // 4 + 2, CH // 4 + 4, CH // 4 + 6, CH // 4 + 8

DELTAS9 = [(dlt, d) for dlt in (-1, 0, 1) for d in range(3)]
SIGD = [(sg, d) for sg in (0, 1) for d in range(3)]
# convT up-pair batches: (first local up-pair, count); even-l use the
# odd-aligned h3 copy, odd-l the standard one
TBATCH = ([(l0, min(2, (NUP - l0 + 1) // 2)) for l0 in range(0, NUP, 4)]
          + [(l0, min(2, (NUP - l0 + 1) // 2)) for l0 in range(1, NUP, 4)])

_CACHE = {}


def _pack_weights(w1, w2, w3, wt, w5):
    f32 = np.float32
    # conv1: m over x-row offset; p = ch + 32*cs (cs = dx); lane = ch'+64*op
    w1c = np.zeros((96, 4, 128), f32)
    for m in range(4):
        for cs in range(3):
            for op in range(2):
                dy = m - op
                if 0 <= dy < 3:
                    w1c[32 * cs:32 * cs + 32, m,
                        64 * op:64 * op + 64] = w1[dy, cs]

    def pack9(wm):
        out = np.zeros((128, 9, 128), f32)
        for k, (dlt, d) in enumerate(DELTAS9):
            for rp in range(2):
                for op in range(2):
                    dy = 2 * dlt + rp - op + 1
                    if 0 <= dy < 3:
                        out[64 * rp:64 * rp + 64, k,
                            64 * op:64 * op + 64] = wm[dy, d]
        return out

    wte = wt[::-1, ::-1]  # jax conv_transpose applies the flipped kernel
    wtc = np.zeros((128, 3, 128), f32)
    for k, dx in enumerate((0, 2, 1)):
        wtc[0:64, k, 0:64] = wte[2, dx]
        wtc[64:128, k, 0:64] = wte[0, dx]
        wtc[64:128, k, 64:128] = wte[1, dx]
    w5c = np.zeros((128, 6, 128), f32)
    for k, (sg, d) in enumerate(SIGD):
        for rp in range(2):
            for op in range(2):
                dy = 2 * sg + rp - op
                if 0 <= dy < 3:
                    w5c[64 * rp:64 * rp + 64, k,
                        64 * op:64 * op + 64] = w5[dy, d]
    return tuple(w.astype(BF) for w in
                 (w1c, pack9(w2), pack9(w3), wtc, w5c))


def _host_prep(features, coors, w1, b1, w2, b2, w3, b3, wt, bt, w5, b5):
    f32 = np.float32
    bi, yi, xi = coors[:, 0], coors[:, 1], coors[:, 2]
    flat = (bi.astype(np.int64) * H + yi) * W + xi
    dense = np.zeros((B * H * W, 32), f32)
    for c in range(32):
        dense[:, c] = np.bincount(flat, weights=features[:, c],
                                  minlength=B * H * W)
    dense = dense.reshape(B, H, W, 32)
    occ = np.bincount(flat, minlength=B * H * W).reshape(B, H, W) > 0
    m0p = np.zeros((B, H + 2, W + 2), bool)
    m0p[:, 1:-1, 1:-1] = occ
    m1 = np.zeros((B, H, W), bool)
    for dy in range(3):
        for dx in range(3):
            m1 |= m0p[:, dy:dy + H, dx:dx + W]
    m4 = np.zeros((B, 2 * H + 1, 2 * W + 1), bool)
    for dy in range(3):
        for dx in range(3):
            m4[:, dy:dy + 2 * H - 1:2, dx:dx + 2 * W - 1:2] |= m1

    w1c, w2c, w3c, wtc, w5c = _pack_weights(w1, w2, w3, wt, w5)
    biases = np.zeros((128, 5), f32)
    for i, bb in enumerate((b1, b2, b3, bt, b5)):
        biases[0:64, i] = bb
        biases[64:128, i] = bb

    in_maps = []
    for core in range(8):
        b, half = core // 2, core % 2
        # padded dense image: rows -8..275, cols -1..258
        Xp = np.zeros((284, 260, 32), f32)
        Xp[8:8 + H, 1:1 + W] = dense[b]
        xs = np.empty((96, 152, NCOL), BF)
        for cs in range(3):
            v = Xp[128 * half + 1:128 * half + 153, cs:cs + NCOL, :]
            xs[32 * cs:32 * cs + 32] = v.transpose(2, 0, 1).astype(BF)
        M1p = np.zeros((272, NCOL), f32)
        M1p[8:8 + H, 1:1 + W] = m1[b]
        m1d = np.empty((128, 70, NCOL), BF)
        for rp in range(2):
            v = M1p[128 * half + 2 + rp:128 * half + 2 + rp + 140:2, :]
            m1d[64 * rp:64 * rp + 64] = np.broadcast_to(
                v.astype(BF)[None], (64, 70, NCOL))
        M4p = np.zeros((520, P4), f32)
        M4p[0:513, 0:513] = m4[b]
        m4d = np.empty((128, 129, P4), BF)
        for rp in range(2):
            v = M4p[256 * half + rp:256 * half + rp + 258:2, :]
            m4d[64 * rp:64 * rp + 64] = np.broadcast_to(
                v.astype(BF)[None], (64, 129, P4))
        in_maps.append(dict(
            xs=np.ascontiguousarray(xs),
            m1d=np.ascontiguousarray(m1d),
            m4d=np.ascontiguousarray(m4d),
            w1c=w1c, w2c=w2c, w3c=w3c, wtc=wtc, w5c=w5c, biases=biases,
        ))
    return in_maps


def _build_program():
    nc = bacc.Bacc("TRN2", target_bir_lowering=False, debug=False,
                   enable_asserts=True, num_devices=8)

    xs_d = nc.dram_tensor("xs", [96, 152, NCOL], BF16,
                          kind="ExternalInput").ap()
    m1_d = nc.dram_tensor("m1d", [128, 70, NCOL], BF16,
                          kind="ExternalInput").ap()
    m4_d = nc.dram_tensor("m4d", [128, 129, P4], BF16,
                          kind="ExternalInput").ap()
    w1_d = nc.dram_tensor("w1c", [96, 4, 128], BF16,
                          kind="ExternalInput").ap()
    w2_d = nc.dram_tensor("w2c", [128, 9, 128], BF16,
                          kind="ExternalInput").ap()
    w3_d = nc.dram_tensor("w3c", [128, 9, 128], BF16,
                          kind="ExternalInput").ap()
    wt_d = nc.dram_tensor("wtc", [128, 3, 128], BF16,
                          kind="ExternalInput").ap()
    w5_d = nc.dram_tensor("w5c", [128, 6, 128], BF16,
                          kind="ExternalInput").ap()
    bias_d = nc.dram_tensor("biases", [128, 5], F32,
                            kind="ExternalInput").ap()
    out_d = nc.dram_tensor("out", [128, 128, 511], BF16,
                           kind="ExternalOutput").ap()

    with tile.TileContext(nc) as tc, ExitStack() as ctx:
        wp = ctx.enter_context(tc.tile_pool(name="wp", bufs=1))
        xp = ctx.enter_context(tc.tile_pool(name="xp", bufs=2))
        mp = ctx.enter_context(tc.tile_pool(name="mp", bufs=2))
        hp = ctx.enter_context(tc.tile_pool(name="hp", bufs=1))
        pp = ctx.enter_context(tc.tile_pool(name="pp", bufs=4, space="PSUM"))
        op = ctx.enter_context(tc.tile_pool(name="op", bufs=4))

        h1 = wp.tile([128, S1, NCOL], BF16, name="h1buf")
        h2 = wp.tile([128, S2, NCOL], BF16, name="h2buf")
        w1t = wp.tile([96, 4, 128], BF16, name="w1t")
        w2t = wp.tile([128, 9, 128], BF16, name="w2t")
        w3t = wp.tile([128, 9, 128], BF16, name="w3t")
        wtt = wp.tile([128, 3, 128], BF16, name="wtt")
        w5t = wp.tile([128, 6, 128], BF16, name="w5t")
        bt = wp.tile([128, 5], F32, name="bt")
        nc.sync.dma_start(w1t[:], w1_d[:])
        nc.sync.dma_start(bt[:], bias_d[:])
        nc.scalar.dma_start(w2t[:], w2_d[:])
        nc.scalar.dma_start(w3t[:], w3_d[:])
        nc.gpsimd.dma_start(wtt[:], wt_d[:])
        nc.gpsimd.dma_start(w5t[:], w5_d[:])

        def conv1_layer(x2, s_lo, m_ch, h_out):
            nc.gpsimd.memset(h_out[:, :, 0:NCOL:NCOL - 1], 0)
            for t0 in range(s_lo, S1, 2):
                nt = min(2, S1 - t0)
                pc = pp.tile([128, 2, 256], F32, name="pc", tag="ps")
                ps = pc[:, 0:nt, :]
                for m in range(4):
                    nc.tensor.matmul(
                        ps, w1t[:, m, :],
                        x2[:, 2 * t0 + m:2 * t0 + m + 2 * nt - 1:2, 0:256],
                        start=(m == 0), stop=(m == 3))
                dst = h_out[:, t0:t0 + nt, 1:257]
                nc.scalar.activation(dst, ps, RELU, bias=bt[:, 0:1])
                nc.vector.tensor_mul(dst, dst, m_ch[:, t0:t0 + nt, 1:257])

        def conv_layer(inp, wt_, deltas, s_lo, nslots, bias_ap, m_ch, moff,
                       h_out):
            nc.gpsimd.memset(h_out[:, :, 0:NCOL:NCOL - 1], 0)
            for t0 in range(s_lo, nslots, 2):
                nt = min(2, nslots - t0)
                pc = pp.tile([128, 2, 256], F32, name="pc", tag="ps")
                ps = pc[:, 0:nt, :]
                for k, (dlt, d) in enumerate(deltas):
                    nc.tensor.matmul(
                        ps, wt_[:, k, :],
                        inp[:, t0 + 1 + dlt:t0 + 1 + dlt + nt, d:d + 256],
                        start=(k == 0), stop=(k == len(deltas) - 1))
                dst = h_out[:, t0:t0 + nt, 1:257]
                nc.scalar.activation(dst, ps, RELU, bias=bias_ap)
                nc.vector.tensor_mul(
                    dst, dst, m_ch[:, t0 + moff:t0 + moff + nt, 1:257])

        NXR = 2 * S1 + 2   # x rows per chunk
        def load_chunk(c):
            x_ch = xp.tile([96, NXR, NCOL], BF16, name="x_ch", tag="x",
                           bufs=1)
            r0 = 2 * SADV * c
            nc.sync.dma_start(x_ch[:, 0:16, :], xs_d[:, r0:r0 + 16, :])
            nc.sync.dma_start(x_ch[:, 16:NXR, :], xs_d[:, r0 + 16:r0 + NXR, :])
            m1_ch = mp.tile([128, S1, NCOL], BF16, name="m1_ch", tag="m1")
            nc.gpsimd.dma_start(m1_ch[:], m1_d[:, SADV * c:SADV * c + S1, :])
            m4_ch = mp.tile([128, NUP, P4], BF16, name="m4_ch", tag="m4",
                            bufs=1)
            nc.scalar.dma_start(m4_ch[:], m4_d[:, (CH // 2) * c:
                                               (CH // 2) * c + NUP, :])
            return x_ch, m1_ch, m4_ch

        def emit_convT(h3, h3o, h4, m4_ch):
            for l0, nb in TBATCH:
                T, ii = (h3o, l0 // 2) if l0 % 2 == 0 else (h3, (l0 + 1) // 2)
                pe = pp.tile([128, 2, 512], F32, name="pe", tag="ps")
                for q in range(nb):
                    nc.tensor.matmul(pe[:, q, 0:257], wtt[:, 0, :],
                                     T[:, ii + q, 1:258],
                                     start=True, stop=False)
                    nc.tensor.matmul(pe[:, q, 0:257], wtt[:, 1, :],
                                     T[:, ii + q, 0:257],
                                     start=False, stop=True)
                de = h4[:, l0:l0 + 2 * nb - 1:2, 0:513:2]
                nc.scalar.activation(de, pe[:, 0:nb, 0:257], RELU,
                                     bias=bt[:, 3:4])
                po = pp.tile([128, 2, 256], F32, name="po", tag="ps")
                nc.tensor.matmul(po[:, 0:nb, :], wtt[:, 2, :],
                                 T[:, ii:ii + nb, 1:257],
                                 start=True, stop=True)
                do = h4[:, l0:l0 + 2 * nb - 1:2, 1:512:2]
                nc.scalar.activation(do, po[:, 0:nb, :], RELU, bias=bt[:, 3:4])
            for p0 in range(0, NUP, (NUP + 3) // 4):
                p1 = min(NUP, p0 + (NUP + 3) // 4)
                nc.vector.tensor_mul(h4[:, p0:p1, 0:513], h4[:, p0:p1, 0:513],
                                     m4_ch[:, p0:p1, 0:513])

        def emit_conv5(h4, c, r0s):
            for r0 in r0s:
                out_sb = op.tile([128, 2, 511], BF16, name="out_sb", tag="o")
                p5 = pp.tile([128, 2, 512], F32, name="p5", tag="ps")
                for q in range(2):
                    for k, (sg, d) in enumerate(SIGD):
                        nc.tensor.matmul(p5[:, q, :], w5t[:, k, :],
                                         h4[:, r0 + q + sg, d:d + 512],
                                         start=(k == 0), stop=(k == 5))
                nc.scalar.activation(out_sb[:], p5[:, :, 0:511], RELU,
                                     bias=bt[:, 4:5])
                nc.gpsimd.dma_start(
                    out_d[:, (CH // 2) * c + r0:(CH // 2) * c + r0 + 2, :],
                    out_sb[:])

        prev = None  # (h3, h3o, h4, m4_ch, c)
        h3prev = None
        nxt = load_chunk(0)
        for it in range(NCH + 1):
            if it < NCH:
                x_ch, m1_ch, m4_ch = nxt
                if it + 1 < NCH:
                    nxt = load_chunk(it + 1)
                h3 = hp.tile([128, S3, NCOL], BF16, name="h3", tag="h3",
                             bufs=2)
                h3o = hp.tile([128, S3 - 1, NCOL], BF16, name="h3o",
                              tag="h3o", bufs=2)
                # carry the exact boundary slots from the previous chunk
                # instead of recomputing them (h1/h2 are persistent tiles;
                # in-place copies between disjoint slot ranges)
                ov1, ov2, ov3 = S1 - SADV, S2 - SADV, S3 - SADV
                if it > 0:
                    nc.vector.tensor_copy(h1[:, 0:ov1, :],
                                          h1[:, SADV:S1, :])
                    nc.vector.tensor_copy(h2[:, 0:ov2, :],
                                          h2[:, SADV:S2, :])
                    nc.vector.tensor_copy(h3[:, 0:ov3, :],
                                          h3prev[:, SADV:S3, :])
                s1, s2, s3 = (ov1, ov2, ov3) if it > 0 else (0, 0, 0)
                conv1_layer(x_ch, s1, m1_ch, h1)
            if prev is not None:
                h3p, h3op, h4p, m4p_, cp = prev
                emit_convT(h3p, h3op, h4p, m4p_)
            if it < NCH:
                conv_layer(h1, w2t, DELTAS9, s2, S2, bt[:, 1:2], m1_ch, 1, h2)
            if prev is not None:
                emit_conv5(h4p, cp, range(0, CH // 4, 2))
            if it < NCH:
                conv_layer(h2, w3t, DELTAS9, s3, S3, bt[:, 2:3], m1_ch, 2, h3)
                nc.vector.tensor_copy(h3o[0:64, :, :],
                                      h3[64:128, 0:S3 - 1, :])
                nc.vector.tensor_copy(h3o[64:128, :, :], h3[0:64, 1:S3, :])
            if prev is not None:
                emit_conv5(h4p, cp, range(CH // 4, CH // 2, 2))
            if it < NCH:
                h4 = hp.tile([128, NUP, P4], BF16, name="h4", tag="h4")
                prev = (h3, h3o, h4, m4_ch, it)
                h3prev = h3
            else:
                prev = None

    nc.compile()
    return nc


def kernel(**inputs):
    features = np.asarray(inputs["features"], np.float32)
    coors = np.asarray(inputs["coors"], np.int32)
    args = [np.asarray(inputs[k], np.float32) for k in
            ("w1", "b1", "w2", "b2", "w3", "b3", "wt", "bt", "w5", "b5")]
    in_maps = _host_prep(features, coors, *args)
    if "nc" not in _CACHE:
        _CACHE["nc"] = _build_program()
    res = run_bass_kernel_spmd(_CACHE["nc"], in_maps,
                               core_ids=list(range(8)), trace=False)
    full = np.zeros((B, 511, 511, 64), np.float32)
    for core in range(8):
        b, half = core // 2, core % 2
        o = np.asarray(res.results[core]["out"]).astype(np.float32)
        rows = o.reshape(2, 64, 128, 511).transpose(2, 0, 3, 1) \
            .reshape(256, 511, 64)
        nrow = 256 if half == 0 else 255
        full[b, 256 * half:256 * half + nrow] = rows[:nrow]
    return full


# revision 20
# speedup vs baseline: 1.9987x; 1.0039x over previous
"""Trainium2 Bass kernel for the sparse-conv network (nn_ExampleNet).

Pipeline (per batch image): scatter 200k sparse voxel features into a dense
[256,256,32] grid, SparseConv(32->64) + 2x SubMConv(64) with an active-site
mask, SparseConvTranspose(64, stride 2), dense 3x3 VALID conv -> [511,511,64].

Strategy: 8-way SPMD (4 batches x 2 row-halves), row-parity-packed layout:
every on-chip tensor stores row pairs across the 128 partitions
(partitions 0:64 = even row channels, 64:128 = odd row), so every matmul
uses all 128 PE output lanes and every activation/mask-multiply runs at
128-partition throughput. Everything stored in bf16 (halves SBUF + DMA +
doubles DVE rate); PSUM accumulation stays fp32.
"""
from contextlib import ExitStack

import numpy as np
import ml_dtypes

import concourse.bacc as bacc
import concourse.mybir as mybir
import concourse.tile as tile
from concourse.bass_utils import run_bass_kernel_spmd

F32 = mybir.dt.float32
F32R = mybir.dt.float32r
BF16 = mybir.dt.bfloat16
RELU = mybir.ActivationFunctionType.Relu
BF = ml_dtypes.bfloat16

B, H, W = 4, 256, 256
NCOL = 258      # padded col count for x/h1-3 slabs
P4 = 516        # h4/m4 col pitch
CH = 64         # output rows per chunk
NCH = 256 // CH
SADV = CH // 4  # x/h slot advance per chunk
NUP = CH // 2 + 1           # h4 up-pairs per chunk
S3, S2, S1, SX = CH // 4 + 2, CH // 4 + 4, CH // 4 + 6, CH // 4 + 8

DELTAS9 = [(dlt, d) for dlt in (-1, 0, 1) for d in range(3)]
SIGD = [(sg, d) for sg in (0, 1) for d in range(3)]
# convT up-pair batches: (first local up-pair, count); even-l use the
# odd-aligned h3 copy, odd-l the standard one
TBATCH = ([(l0, min(2, (NUP - l0 + 1) // 2)) for l0 in range(0, NUP, 4)]
          + [(l0, min(2, (NUP - l0 + 1) // 2)) for l0 in range(1, NUP, 4)])

_CACHE = {}


def _pack_weights(w1, w2, w3, wt, w5):
    f32 = np.float32
    # conv1: m over x-row offset; p = ch + 32*cs (cs = dx); lane = ch'+64*op
    w1c = np.zeros((96, 4, 128), f32)
    for m in range(4):
        for cs in range(3):
            for op in range(2):
                dy = m - op
                if 0 <= dy < 3:
                    w1c[32 * cs:32 * cs + 32, m,
                        64 * op:64 * op + 64] = w1[dy, cs]

    def pack9(wm):
        out = np.zeros((128, 9, 128), f32)
        for k, (dlt, d) in enumerate(DELTAS9):
            for rp in range(2):
                for op in range(2):
                    dy = 2 * dlt + rp - op + 1
                    if 0 <= dy < 3:
                        out[64 * rp:64 * rp + 64, k,
                            64 * op:64 * op + 64] = wm[dy, d]
        return out

    wte = wt[::-1, ::-1]  # jax conv_transpose applies the flipped kernel
    wtc = np.zeros((128, 3, 128), f32)
    for k, dx in enumerate((0, 2, 1)):
        wtc[0:64, k, 0:64] = wte[2, dx]
        wtc[64:128, k, 0:64] = wte[0, dx]
        wtc[64:128, k, 64:128] = wte[1, dx]
    w5c = np.zeros((128, 6, 128), f32)
    for k, (sg, d) in enumerate(SIGD):
        for rp in range(2):
            for op in range(2):
                dy = 2 * sg + rp - op
                if 0 <= dy < 3:
                    w5c[64 * rp:64 * rp + 64, k,
                        64 * op:64 * op + 64] = w5[dy, d]
    return tuple(w.astype(BF) for w in
                 (w1c, pack9(w2), pack9(w3), wtc, w5c))


def _host_prep(features, coors, w1, b1, w2, b2, w3, b3, wt, bt, w5, b5):
    f32 = np.float32
    bi, yi, xi = coors[:, 0], coors[:, 1], coors[:, 2]
    flat = (bi.astype(np.int64) * H + yi) * W + xi
    dense = np.zeros((B * H * W, 32), f32)
    for c in range(32):
        dense[:, c] = np.bincount(flat, weights=features[:, c],
                                  minlength=B * H * W)
    dense = dense.reshape(B, H, W, 32)
    occ = np.bincount(flat, minlength=B * H * W).reshape(B, H, W) > 0
    m0p = np.zeros((B, H + 2, W + 2), bool)
    m0p[:, 1:-1, 1:-1] = occ
    m1 = np.zeros((B, H, W), bool)
    for dy in range(3):
        for dx in range(3):
            m1 |= m0p[:, dy:dy + H, dx:dx + W]
    m4 = np.zeros((B, 2 * H + 1, 2 * W + 1), bool)
    for dy in range(3):
        for dx in range(3):
            m4[:, dy:dy + 2 * H - 1:2, dx:dx + 2 * W - 1:2] |= m1

    w1c, w2c, w3c, wtc, w5c = _pack_weights(w1, w2, w3, wt, w5)
    biases = np.zeros((128, 5), f32)
    for i, bb in enumerate((b1, b2, b3, bt, b5)):
        biases[0:64, i] = bb
        biases[64:128, i] = bb

    in_maps = []
    for core in range(8):
        b, half = core // 2, core % 2
        # padded dense image: rows -8..275, cols -1..258
        Xp = np.zeros((284, 260, 32), f32)
        Xp[8:8 + H, 1:1 + W] = dense[b]
        xs = np.empty((96, 152, NCOL), BF)
        for cs in range(3):
            v = Xp[128 * half + 1:128 * half + 153, cs:cs + NCOL, :]
            xs[32 * cs:32 * cs + 32] = v.transpose(2, 0, 1).astype(BF)
        M1p = np.zeros((272, NCOL), f32)
        M1p[8:8 + H, 1:1 + W] = m1[b]
        m1d = np.empty((128, 70, NCOL), BF)
        for rp in range(2):
            v = M1p[128 * half + 2 + rp:128 * half + 2 + rp + 140:2, :]
            m1d[64 * rp:64 * rp + 64] = np.broadcast_to(
                v.astype(BF)[None], (64, 70, NCOL))
        M4p = np.zeros((520, P4), f32)
        M4p[0:513, 0:513] = m4[b]
        m4d = np.empty((128, 129, P4), BF)
        for rp in range(2):
            v = M4p[256 * half + rp:256 * half + rp + 258:2, :]
            m4d[64 * rp:64 * rp + 64] = np.broadcast_to(
                v.astype(BF)[None], (64, 129, P4))
        in_maps.append(dict(
            xs=np.ascontiguousarray(xs),
            m1d=np.ascontiguousarray(m1d),
            m4d=np.ascontiguousarray(m4d),
            w1c=w1c, w2c=w2c, w3c=w3c, wtc=wtc, w5c=w5c, biases=biases,
        ))
    return in_maps


def _build_program():
    nc = bacc.Bacc("TRN2", target_bir_lowering=False, debug=False,
                   enable_asserts=True, num_devices=8)

    xs_d = nc.dram_tensor("xs", [96, 152, NCOL], BF16,
                          kind="ExternalInput").ap()
    m1_d = nc.dram_tensor("m1d", [128, 70, NCOL], BF16,
                          kind="ExternalInput").ap()
    m4_d = nc.dram_tensor("m4d", [128, 129, P4], BF16,
                          kind="ExternalInput").ap()
    w1_d = nc.dram_tensor("w1c", [96, 4, 128], BF16,
                          kind="ExternalInput").ap()
    w2_d = nc.dram_tensor("w2c", [128, 9, 128], BF16,
                          kind="ExternalInput").ap()
    w3_d = nc.dram_tensor("w3c", [128, 9, 128], BF16,
                          kind="ExternalInput").ap()
    wt_d = nc.dram_tensor("wtc", [128, 3, 128], BF16,
                          kind="ExternalInput").ap()
    w5_d = nc.dram_tensor("w5c", [128, 6, 128], BF16,
                          kind="ExternalInput").ap()
    bias_d = nc.dram_tensor("biases", [128, 5], F32,
                            kind="ExternalInput").ap()
    out_d = nc.dram_tensor("out", [128, 128, 511], BF16,
                           kind="ExternalOutput").ap()

    with tile.TileContext(nc) as tc, ExitStack() as ctx:
        wp = ctx.enter_context(tc.tile_pool(name="wp", bufs=1))
        xp = ctx.enter_context(tc.tile_pool(name="xp", bufs=2))
        mp = ctx.enter_context(tc.tile_pool(name="mp", bufs=2))
        hp = ctx.enter_context(tc.tile_pool(name="hp", bufs=1))
        pp = ctx.enter_context(tc.tile_pool(name="pp", bufs=4, space="PSUM"))
        op = ctx.enter_context(tc.tile_pool(name="op", bufs=4))

        h1 = wp.tile([128, S1, NCOL], BF16, name="h1buf")
        h2 = wp.tile([128, S2, NCOL], BF16, name="h2buf")
        w1t = wp.tile([96, 4, 128], BF16, name="w1t")
        w2t = wp.tile([128, 9, 128], BF16, name="w2t")
        w3t = wp.tile([128, 9, 128], BF16, name="w3t")
        wtt = wp.tile([128, 3, 128], BF16, name="wtt")
        w5t = wp.tile([128, 6, 128], BF16, name="w5t")
        bt = wp.tile([128, 5], F32, name="bt")
        nc.sync.dma_start(w1t[:], w1_d[:])
        nc.sync.dma_start(bt[:], bias_d[:])
        nc.scalar.dma_start(w2t[:], w2_d[:])
        nc.scalar.dma_start(w3t[:], w3_d[:])
        nc.scalar.dma_start(wtt[:], wt_d[:])
        nc.scalar.dma_start(w5t[:], w5_d[:])

        def conv1_layer(x2, s_lo, m_ch, h_out):
            nc.gpsimd.memset(h_out[:, :, 0:NCOL:NCOL - 1], 0)
            for t0 in range(s_lo, S1, 2):
                nt = min(2, S1 - t0)
                pc = pp.tile([128, 2, 256], F32, name="pc", tag="ps")
                ps = pc[:, 0:nt, :]
                for m in range(4):
                    nc.tensor.matmul(
                        ps, w1t[:, m, :],
                        x2[:, 2 * t0 + m:2 * t0 + m + 2 * nt - 1:2, 0:256],
                        start=(m == 0), stop=(m == 3))
                dst = h_out[:, t0:t0 + nt, 1:257]
                nc.scalar.activation(dst, ps, RELU, bias=bt[:, 0:1])
                nc.vector.tensor_mul(dst, dst, m_ch[:, t0:t0 + nt, 1:257])

        def conv_layer(inp, wt_, deltas, s_lo, nslots, bias_ap, m_ch, moff,
                       h_out):
            nc.gpsimd.memset(h_out[:, :, 0:NCOL:NCOL - 1], 0)
            for t0 in range(s_lo, nslots, 2):
                nt = min(2, nslots - t0)
                pc = pp.tile([128, 2, 256], F32, name="pc", tag="ps")
                ps = pc[:, 0:nt, :]
                for k, (dlt, d) in enumerate(deltas):
                    nc.tensor.matmul(
                        ps, wt_[:, k, :],
                        inp[:, t0 + 1 + dlt:t0 + 1 + dlt + nt, d:d + 256],
                        start=(k == 0), stop=(k == len(deltas) - 1))
                dst = h_out[:, t0:t0 + nt, 1:257]
                nc.scalar.activation(dst, ps, RELU, bias=bias_ap)
                nc.vector.tensor_mul(
                    dst, dst, m_ch[:, t0 + moff:t0 + moff + nt, 1:257])

        NXR = 2 * S1 + 2   # x rows per chunk
        def load_chunk(c):
            x_ch = xp.tile([96, NXR, NCOL], BF16, name="x_ch", tag="x",
                           bufs=1)
            r0 = 2 * SADV * c
            nc.sync.dma_start(x_ch[:, 0:16, :], xs_d[:, r0:r0 + 16, :])
            nc.sync.dma_start(x_ch[:, 16:NXR, :], xs_d[:, r0 + 16:r0 + NXR, :])
            m1_ch = mp.tile([128, S1, NCOL], BF16, name="m1_ch", tag="m1")
            nc.gpsimd.dma_start(m1_ch[:, 0:8, :],
                                m1_d[:, SADV * c:SADV * c + 8, :])
            nc.gpsimd.dma_start(m1_ch[:, 8:S1, :],
                                m1_d[:, SADV * c + 8:SADV * c + S1, :])
            return x_ch, m1_ch

        def load_m4(c):
            # deliberately emitted after conv1 so its bulk transfer queues
            # behind conv1's activations and never starves the startup loads
            m4_ch = mp.tile([128, NUP, P4], BF16, name="m4_ch", tag="m4",
                            bufs=1)
            nc.scalar.dma_start(m4_ch[:], m4_d[:, (CH // 2) * c:
                                               (CH // 2) * c + NUP, :])
            return m4_ch

        def emit_convT(h3, h3o, h4, m4_ch):
            for l0, nb in TBATCH:
                T, ii = (h3o, l0 // 2) if l0 % 2 == 0 else (h3, (l0 + 1) // 2)
                pe = pp.tile([128, 2, 512], F32, name="pe", tag="ps")
                for q in range(nb):
                    nc.tensor.matmul(pe[:, q, 0:257], wtt[:, 0, :],
                                     T[:, ii + q, 1:258],
                                     start=True, stop=False)
                    nc.tensor.matmul(pe[:, q, 0:257], wtt[:, 1, :],
                                     T[:, ii + q, 0:257],
                                     start=False, stop=True)
                de = h4[:, l0:l0 + 2 * nb - 1:2, 0:513:2]
                nc.scalar.activation(de, pe[:, 0:nb, 0:257], RELU,
                                     bias=bt[:, 3:4])
                po = pp.tile([128, 2, 256], F32, name="po", tag="ps")
                nc.tensor.matmul(po[:, 0:nb, :], wtt[:, 2, :],
                                 T[:, ii:ii + nb, 1:257],
                                 start=True, stop=True)
                do = h4[:, l0:l0 + 2 * nb - 1:2, 1:512:2]
                nc.scalar.activation(do, po[:, 0:nb, :], RELU, bias=bt[:, 3:4])
            for p0 in range(0, NUP, (NUP + 3) // 4):
                p1 = min(NUP, p0 + (NUP + 3) // 4)
                nc.vector.tensor_mul(h4[:, p0:p1, 0:513], h4[:, p0:p1, 0:513],
                                     m4_ch[:, p0:p1, 0:513])

        def emit_conv5(h4, c, r0s):
            for r0 in r0s:
                out_sb = op.tile([128, 2, 511], BF16, name="out_sb", tag="o")
                p5 = pp.tile([128, 2, 512], F32, name="p5", tag="ps")
                for q in range(2):
                    for k, (sg, d) in enumerate(SIGD):
                        nc.tensor.matmul(p5[:, q, :], w5t[:, k, :],
                                         h4[:, r0 + q + sg, d:d + 512],
                                         start=(k == 0), stop=(k == 5))
                nc.scalar.activation(out_sb[:], p5[:, :, 0:511], RELU,
                                     bias=bt[:, 4:5])
                nc.gpsimd.dma_start(
                    out_d[:, (CH // 2) * c + r0:(CH // 2) * c + r0 + 2, :],
                    out_sb[:])

        prev = None  # (h3, h3o, h4, m4_ch, c)
        h3prev = None
        nxt = load_chunk(0)
        for it in range(NCH + 1):
            if it < NCH:
                x_ch, m1_ch = nxt
                if it + 1 < NCH:
                    nxt = load_chunk(it + 1)
                h3 = hp.tile([128, S3, NCOL], BF16, name="h3", tag="h3",
                             bufs=2)
                h3o = hp.tile([128, S3 - 1, NCOL], BF16, name="h3o",
                              tag="h3o", bufs=2)
                # carry the exact boundary slots from the previous chunk
                # instead of recomputing them (h1/h2 are persistent tiles;
                # in-place copies between disjoint slot ranges)
                ov1, ov2, ov3 = S1 - SADV, S2 - SADV, S3 - SADV
                if it > 0:
                    nc.vector.tensor_copy(h1[:, 0:ov1, :],
                                          h1[:, SADV:S1, :])
                    nc.vector.tensor_copy(h2[:, 0:ov2, :],
                                          h2[:, SADV:S2, :])
                    nc.vector.tensor_copy(h3[:, 0:ov3, :],
                                          h3prev[:, SADV:S3, :])
                s1, s2, s3 = (ov1, ov2, ov3) if it > 0 else (0, 0, 0)
                conv1_layer(x_ch, s1, m1_ch, h1)
                m4_ch = load_m4(it)
            if prev is not None:
                h3p, h3op, h4p, m4p_, cp = prev
                emit_convT(h3p, h3op, h4p, m4p_)
            if it < NCH:
                conv_layer(h1, w2t, DELTAS9, s2, S2, bt[:, 1:2], m1_ch, 1, h2)
            if prev is not None:
                emit_conv5(h4p, cp, range(0, CH // 4, 2))
            if it < NCH:
                conv_layer(h2, w3t, DELTAS9, s3, S3, bt[:, 2:3], m1_ch, 2, h3)
                nc.vector.tensor_copy(h3o[0:64, :, :],
                                      h3[64:128, 0:S3 - 1, :])
                nc.vector.tensor_copy(h3o[64:128, :, :], h3[0:64, 1:S3, :])
            if prev is not None:
                emit_conv5(h4p, cp, range(CH // 4, CH // 2, 2))
            if it < NCH:
                h4 = hp.tile([128, NUP, P4], BF16, name="h4", tag="h4")
                prev = (h3, h3o, h4, m4_ch, it)
                h3prev = h3
            else:
                prev = None

    nc.compile()
    return nc


def kernel(**inputs):
    features = np.asarray(inputs["features"], np.float32)
    coors = np.asarray(inputs["coors"], np.int32)
    args = [np.asarray(inputs[k], np.float32) for k in
            ("w1", "b1", "w2", "b2", "w3", "b3", "wt", "bt", "w5", "b5")]
    in_maps = _host_prep(features, coors, *args)
    if "nc" not in _CACHE:
        _CACHE["nc"] = _build_program()
    res = run_bass_kernel_spmd(_CACHE["nc"], in_maps,
                               core_ids=list(range(8)), trace=False)
    full = np.zeros((B, 511, 511, 64), np.float32)
    for core in range(8):
        b, half = core // 2, core % 2
        o = np.asarray(res.results[core]["out"]).astype(np.float32)
        rows = o.reshape(2, 64, 128, 511).transpose(2, 0, 3, 1) \
            .reshape(256, 511, 64)
        nrow = 256 if half == 0 else 255
        full[b, 256 * half:256 * half + nrow] = rows[:nrow]
    return full
